# revision 2
# baseline (speedup 1.0000x reference)
"""3-layer GAT (PyG GATConv semantics) on 8 Trainium2 NeuronCores — v2.

Strategy (dst-sharded, big-batch dma_gather):
- Nodes assigned to 160 degree-balanced blocks of <=128 dst slots; 20 blocks/core.
- Per layer l in {1,2}: dense phase computes h_aug = hprev @ [W | W@a_dst] per block
  (f16 matmuls), writes h rows (512B f16) to the core's table shard + alpha_dst rows
  (256B-padded) to a LOCAL table; AllGather of the h shard only.
- Edge phase per 2-block chunk (4352 edge slots): ONE dma_gather of h rows by edge src
  (512B/row) + ONE dma_gather of alpha_dst rows by edge dst (256B/row, local HBM).
  alpha_src per edge = reduce(hg * a_src) on DVE. e=leaky(as+ad), ex=exp(e) (f32->f16),
  messages m = ex*hg; aggregation + softmax denominators via one PE matmul per
  128-edge tile (lhsT = S 0/1 selector built by iota-compare, rhs = [m | ex]).
- Layer 3 (heads=1, C=1): h3/alpha_src3 in a 256B-padded global table (AllGather'd),
  alpha_dst3 local; same gather scheme; tiny aggregation matmuls.
- dma_gather requires single_packet=False here and the mlp Q7 library
  (load_library + codegen_inst_isa_subclasses to materialize the reload's ISA bytes).
- Gather index layout: flat position j = (p%16)*(N/16) + p//16 + 8*t maps to output
  slot (partition p, tile t); host pre-wraps to [16, N/16] rows tiled x8.

The walrus in this toolchain accepts only ONE sync wait per instruction; BassOneWait
splits Tile-generated multi-waits into single-wait EventSemaphore ops at serialization.
"""
import numpy as np
from contextlib import ExitStack
import heapq

import orjson
import concourse.bass as bass
import concourse.tile as tile
from concourse import mybir
from concourse.bass_utils import run_bass_kernel_spmd
from concourse.library_config import mlp

# problem constants (fixed by the harness's setup_inputs)
N_NODES = 20000
N_EDGES = 320000
IN_DIM = 128
HID = 64
HEADS = 4
HC = HEADS * HID          # 256
WAUG = HC + HEADS         # 260 = [W | W@a_dst]
NEG = 0.2
NCORES = 8
P = 128
NBLK = 20                 # dst blocks per core
SLOTS = NBLK * P          # 2560 slots per core
TOT_SLOTS = SLOTS * NCORES
CBLK = 2                  # blocks per gather chunk
NCHUNK = NBLK // CBLK
ADW = 128                 # alpha_dst table row width (f16) -> 256B rows

F32 = mybir.dt.float32
F16 = mybir.dt.float16
I16 = mybir.dt.int16

EXPF = mybir.ActivationFunctionType.Exp


def _split_multiwaits(bir: bytes) -> bytes:
    """Walrus here allows only 1 sync wait per instruction -> hoist extras onto
    same-engine EventSemaphore waits (dedup repeated ge-waits per engine; sems
    are monotonic within the block, so a repeated >= wait is a no-op)."""
    j = orjson.loads(bir)
    ctr = 0
    for fn in j["functions"]:
        for blk in fn["blocks"]:
            out_l = []
            last_wait = {}   # engine -> set of (id, value) already waited at this point
            for ins in blk["instructions"]:
                eng = ins.get("engine")
                si = ins.get("sync_info")
                ow = (si or {}).get("on_wait") or []
                keep = 1
                if len(ow) > keep:
                    seen = last_wait.setdefault(eng, set())
                    for w in ow[:len(ow) - keep]:
                        key = (w.get("id"), w.get("wait_mode"), w.get("wait_value"))
                        if w.get("wait_mode") == "sem-ge-imm":
                            if key in seen:
                                continue
                            seen.add(key)
                        ctr += 1
                        out_l.append({
                            "engine": eng, "ins": [], "outs": [],
                            "name": f"mwsplit-{ctr}", "opcode": "EventSemaphore",
                            "sync_info": {"on_update": [], "on_wait": [w]},
                        })
                    si["on_wait"] = ow[len(ow) - keep:]
                out_l.append(ins)
            blk["instructions"] = out_l
    return orjson.dumps(j)


class BassOneWait(bass.Bass):
    def to_json_bytes(self):
        return _split_multiwaits(super().to_json_bytes())


# ---------------------------------------------------------------- host prep

def _wrap_idx(rows, T):
    """rows [ntiles, 128] of source-row ids (output slot (p, tile)) ->
    dma_gather idx tensor chunks concatenated: [128, NCHUNK*CBLK*T*8] int16."""
    ntiles = rows.shape[0]
    per_chunk = CBLK * T
    chunks = []
    for c in range(ntiles // per_chunk):
        r = rows[c * per_chunk:(c + 1) * per_chunk]      # [per_chunk, 128]
        N = per_chunk * P
        flat = np.zeros(N, np.int64)
        TL, Pp = np.meshgrid(np.arange(per_chunk), np.arange(P), indexing="ij")
        jj = (Pp % 16) * (N // 16) + (Pp // 16) + 8 * TL
        flat[jj] = r[TL, Pp]
        chunks.append(flat.reshape(16, N // 16))
    allc = np.concatenate(chunks, axis=1)
    return np.tile(allc, (8, 1)).astype(np.int16)


def _preprocess(edge_index):
    """Assign nodes to degree-balanced blocks; build per-core edge arrays."""
    src = np.asarray(edge_index[0], dtype=np.int64)
    dst = np.asarray(edge_index[1], dtype=np.int64)
    loops = np.arange(N_NODES, dtype=np.int64)
    src = np.concatenate([src, loops])
    dst = np.concatenate([dst, loops])
    deg = np.bincount(dst, minlength=N_NODES).astype(np.int64)

    NB_TOT = NCORES * NBLK
    order = np.argsort(-deg, kind="stable")
    blk_of = np.empty(N_NODES, np.int32)
    slot_of = np.empty(N_NODES, np.int32)
    heap = [(0, 0, b) for b in range(NB_TOT)]
    heapq.heapify(heap)
    cnt = np.zeros(NB_TOT, np.int32)
    load = np.zeros(NB_TOT, np.int64)
    for n in order:
        while True:
            l, _, b = heapq.heappop(heap)
            if cnt[b] < P:
                break
        blk_of[n] = b
        slot_of[n] = cnt[b]
        cnt[b] += 1
        load[b] += deg[n]
        if cnt[b] < P:
            heapq.heappush(heap, (load[b], cnt[b], b))

    T = int(np.ceil(load.max() / P))  # edge tiles per block (same for all)
    gslot = blk_of.astype(np.int64) * P + slot_of        # global table row of node
    node_of_slot = np.full(NB_TOT * P, -1, np.int64)
    node_of_slot[gslot] = np.arange(N_NODES)

    # bucket edges by dst block, then sort by src gslot for HBM locality
    eb = blk_of[dst]
    order_e = np.lexsort((gslot[src], eb))
    src_s = src[order_e]
    dst_s = dst[order_e]
    eb_s = eb[order_e]
    starts = np.searchsorted(eb_s, np.arange(NB_TOT + 1))

    NT = NBLK * T
    srcg_idx = np.zeros((NCORES, P, NCHUNK * CBLK * T * 8), np.int16)
    dstl_idx = np.zeros_like(srcg_idx)
    dblk = np.full((NCORES, P, NT), -1.0, np.float16)  # block-local dst slot (-1 pad)
    for c in range(NCORES):
        rows_s = np.zeros((NT, P), np.int64)
        rows_d = np.zeros((NT, P), np.int64)
        for lb in range(NBLK):
            b = c * NBLK + lb
            e0, e1 = starts[b], starts[b + 1]
            k = e1 - e0
            col = np.zeros(T * P, np.int64)
            col[:k] = gslot[src_s[e0:e1]]
            rows_s[lb * T:(lb + 1) * T] = col.reshape(T, P)
            col_d = np.zeros(T * P, np.int64)
            col_d[:k] = lb * P + slot_of[dst_s[e0:e1]]
            rows_d[lb * T:(lb + 1) * T] = col_d.reshape(T, P)
            col_b = np.full(T * P, -1.0, np.float32)
            col_b[:k] = slot_of[dst_s[e0:e1]]
            dblk[c, :, lb * T:(lb + 1) * T] = col_b.reshape(T, P).T.astype(np.float16)
        srcg_idx[c] = _wrap_idx(rows_s, T)
        dstl_idx[c] = _wrap_idx(rows_d, T)

    return T, gslot, node_of_slot, srcg_idx, dstl_idx, dblk


def _aug_weights(W, a_dst, heads, hid):
    """[W | wd] with wd[:,h] = W[:, h*hid:(h+1)*hid] @ a_dst[h]; f16."""
    cin = W.shape[0]
    wd = np.zeros((cin, heads), np.float32)
    for h in range(heads):
        wd[:, h] = W[:, h * hid:(h + 1) * hid] @ a_dst[h]
    return np.concatenate([W, wd], axis=1).astype(np.float16)


# ---------------------------------------------------------------- device kernel

def _build(T):
    NT = NBLK * T
    CT = CBLK * T            # tiles per chunk
    NIDX = CT * P            # gather indices per chunk
    IW = NIDX // 16          # idx cols per chunk
    nc = BassOneWait()
    dp = nc.declare_dram_parameter
    xT_in = dp("xT_in", [P, NBLK * P], F16, isOutput=False)
    srcg_in = dp("srcg_in", [P, NCHUNK * IW], I16, isOutput=False)
    dstl_in = dp("dstl_in", [P, NCHUNK * IW], I16, isOutput=False)
    dblk_in = dp("dblk_in", [P, NT], F16, isOutput=False)
    wa1_in = dp("wa1_in", [IN_DIM, WAUG], F16, isOutput=False)
    wa2_in = dp("wa2_in", [HC, WAUG], F16, isOutput=False)
    ws1_in = dp("ws1_in", [1, HC], F16, isOutput=False)
    ws2_in = dp("ws2_in", [1, HC], F16, isOutput=False)
    w3_in = dp("w3_in", [1, HC], F16, isOutput=False)
    sc3_in = dp("sc3_in", [1, 4], F32, isOutput=False)   # a_src3, a_dst3, b3, 0
    b1_in = dp("b1_in", [1, HC], F32, isOutput=False)
    b2_in = dp("b2_in", [1, HC], F32, isOutput=False)
    iota_in = dp("iota_in", [1, P], F16, isOutput=False)
    ident_in = dp("ident_in", [P, P], F16, isOutput=False)
    out_p = dp("out_p", [P, NBLK], F32, isOutput=True)

    # internal DRAM
    tab_sh = [nc.dram_tensor(f"tab_sh{l}", [SLOTS, HC], F16) for l in (1, 2)]
    tab_full = [nc.dram_tensor(f"tab_full{l}", [TOT_SLOTS, HC], F16) for l in (1, 2)]
    ad_t = [nc.dram_tensor(f"ad{l}", [SLOTS, ADW], F16) for l in (1, 2, 3)]
    tab3_sh = nc.dram_tensor("tab3_sh", [SLOTS, ADW], F16)
    tab3_full = nc.dram_tensor("tab3_full", [TOT_SLOTS, ADW], F16)

    groups = [list(range(NCORES))]

    with tile.TileContext(nc) as tc, ExitStack() as ctx:
        consts = ctx.enter_context(tc.tile_pool(name="consts", bufs=1))
        meta = ctx.enter_context(tc.tile_pool(name="meta", bufs=1))
        spool = ctx.enter_context(tc.tile_pool(name="spool", bufs=2))
        gpool = ctx.enter_context(tc.tile_pool(name="gpool", bufs=2))
        apool = ctx.enter_context(tc.tile_pool(name="apool", bufs=2))
        mpool = ctx.enter_context(tc.tile_pool(name="mpool", bufs=2))
        small = ctx.enter_context(tc.tile_pool(name="small", bufs=2))
        psd = ctx.enter_context(tc.tile_pool(name="psd", bufs=2, space="PSUM"))
        pse = ctx.enter_context(tc.tile_pool(name="pse", bufs=2, space="PSUM"))
        pst = ctx.enter_context(tc.tile_pool(name="pst", bufs=2, space="PSUM"))

        nc.gpsimd.load_library(mlp)
        nidx_reg = nc.gpsimd.to_reg(CBLK * T * P)

        # ---- constants / metadata
        ident16 = consts.tile([P, P], F16)
        nc.sync.dma_start(out=ident16, in_=ident_in[:])
        wa1 = consts.tile([P, WAUG], F16)
        nc.sync.dma_start(out=wa1, in_=wa1_in[:])
        wa2 = consts.tile([P, 2, WAUG], F16)
        nc.sync.dma_start(out=wa2, in_=wa2_in.rearrange("(j p) a -> p j a", p=P))

        def rep_load(name, src, n, dt):
            t = consts.tile([P, n], dt, tag=name)
            bc = bass.AP(tensor=src.tensor, offset=0, ap=[[0, P], [1, n]])
            nc.sync.dma_start(out=t, in_=bc)
            return t
        ws1 = rep_load("ws1", ws1_in[:], HC, F16)
        ws2 = rep_load("ws2", ws2_in[:], HC, F16)
        w3r = rep_load("w3r", w3_in[:], HC, F16)
        sc3 = rep_load("sc3", sc3_in[:], 4, F32)
        b1r = rep_load("b1r", b1_in[:], HC, F32)
        b2r = rep_load("b2r", b2_in[:], HC, F32)
        iot = rep_load("iot", iota_in[:], P, F16)

        dblk = meta.tile([P, NT], F16)
        nc.sync.dma_start(out=dblk, in_=dblk_in[:])
        srcg = meta.tile([P, NCHUNK * IW], I16)
        nc.sync.dma_start(out=srcg, in_=srcg_in[:])
        dstl = meta.tile([P, NCHUNK * IW], I16)
        nc.sync.dma_start(out=dstl, in_=dstl_in[:])
        hT = meta.tile([P, 2 * NBLK, P], F16)
        nc.sync.dma_start(out=hT[:, 0:NBLK, :],
                          in_=xT_in.rearrange("p (b n) -> p b n", n=P))
        outsb = meta.tile([P, NBLK], F32)

        def bcast_row(t, shape):
            ap = [list(t.ap[0])]
            for s in shape[1:-1]:
                ap.append([0, s])
            ap.append([t.ap[-1][0], shape[-1]])
            return bass.AP(tensor=t.tensor, offset=t.offset, ap=ap)

        def dense_block(lidx, b):
            """project block b for layer lidx (0/1): tab rows + alpha_dst rows."""
            ps = psd.tile([P, WAUG], F32, tag="dense")
            if lidx == 0:
                nc.tensor.matmul(ps, hT[:, b, :], wa1, start=True, stop=True)
            else:
                nc.tensor.matmul(ps, hT[:, 2 * b, :], wa2[:, 0, :],
                                 start=True, stop=False)
                nc.tensor.matmul(ps, hT[:, 2 * b + 1, :], wa2[:, 1, :],
                                 start=False, stop=True)
            tabt = small.tile([P, HC], F16, tag="tabt")
            nc.vector.tensor_copy(out=tabt, in_=ps[:, 0:HC])
            nc.sync.dma_start(
                out=tab_sh[lidx].rearrange("(b p) a -> p b a", p=P)[:, b, :],
                in_=tabt)
            adt = small.tile([P, HEADS], F16, tag="adt")
            nc.vector.tensor_copy(out=adt, in_=ps[:, HC:WAUG])
            nc.sync.dma_start(
                out=ad_t[lidx].rearrange("(b p) a -> p b a", p=P)[:, b, 0:HEADS],
                in_=adt)

        def build_s(c):
            """S selector [P, CT, P] f16 for chunk c: S[e, tl, slot]=(dblk==slot)."""
            S = spool.tile([P, CT, P], F16, tag="S")
            sl = slice(c * CT, (c + 1) * CT)
            db_b = bass.AP(tensor=dblk.tensor, offset=dblk[:, sl].offset,
                           ap=[dblk.ap[0], [dblk.ap[1][0], CT], [0, P]])
            nc.vector.tensor_tensor(out=S, in0=db_b,
                                    in1=bcast_row(iot, [P, CT, P]),
                                    op=mybir.AluOpType.is_equal)
            return S

        def edge12(lidx, ws, brow, after_block):
            """edge phase for layer lidx in {0,1}; after_block(b, hn16) hook."""
            for c in range(NCHUNK):
                hg = gpool.tile([P, CT, HC], F16, tag="hg")
                isl = slice(c * IW, (c + 1) * IW)
                nc.gpsimd.dma_gather(hg, tab_full[lidx][:], srcg[:, isl],
                                     NIDX, nidx_reg, HC, single_packet=False)
                ad = apool.tile([P, CT, ADW], F16, tag="ad")
                nc.gpsimd.dma_gather(ad, ad_t[lidx][:], dstl[:, isl],
                                     NIDX, nidx_reg, ADW, single_packet=False)
                S = build_s(c)
                for bi in range(CBLK):
                    b = c * CBLK + bi
                    tsl = slice(bi * T, (bi + 1) * T)
                    hgb = hg[:, tsl, :]
                    m = mpool.tile([P, T, HC + HEADS], F16, tag="m")
                    # alpha_src = reduce(hg * a_src) over channels within head
                    nc.vector.tensor_tensor(
                        out=m[:, :, 0:HC].rearrange("p t (h k) -> p t h k", h=HEADS),
                        in0=hgb.rearrange("p t (h k) -> p t h k", h=HEADS),
                        in1=bcast_row(ws, [P, T, HC]).rearrange(
                            "p t (h k) -> p t h k", h=HEADS),
                        op=mybir.AluOpType.mult)
                    als = small.tile([P, T, HEADS], F32, tag="als")
                    nc.vector.tensor_reduce(
                        out=als, in_=m[:, :, 0:HC].rearrange(
                            "p t (h k) -> p t h k", h=HEADS),
                        axis=mybir.AxisListType.X, op=mybir.AluOpType.add)
                    nc.vector.tensor_tensor(out=als, in0=als,
                                            in1=ad[:, tsl, 0:HEADS],
                                            op=mybir.AluOpType.add)
                    lk = small.tile([P, T, HEADS], F32, tag="lk")
                    nc.vector.tensor_scalar_mul(lk, als, NEG)
                    nc.vector.tensor_tensor(out=lk, in0=lk, in1=als,
                                            op=mybir.AluOpType.max)
                    exf = small.tile([P, T, HEADS], F16, tag="exf")
                    nc.scalar.activation(out=exf, in_=lk, func=EXPF)
                    ex_b = bass.AP(tensor=exf.tensor, offset=exf.offset,
                                   ap=[exf.ap[0], exf.ap[1], exf.ap[2], [0, HID]])
                    nc.vector.tensor_tensor(
                        out=m[:, :, 0:HC].rearrange("p t (h k) -> p t h k", h=HEADS),
                        in0=hgb.rearrange("p t (h k) -> p t h k", h=HEADS),
                        in1=ex_b, op=mybir.AluOpType.mult)
                    nc.vector.tensor_copy(out=m[:, :, HC:HC + HEADS], in_=exf)

                    ps = pse.tile([P, HC + HEADS], F32, tag="agg")
                    for t in range(T):
                        nc.tensor.matmul(ps, S[:, bi * T + t, :], m[:, t, :],
                                         start=(t == 0), stop=(t == T - 1))

                    den = small.tile([P, HEADS], F32, tag="den")
                    nc.vector.tensor_scalar_max(den, ps[:, HC:HC + HEADS], 1e-30)
                    rec = small.tile([P, HEADS], F32, tag="rec")
                    nc.vector.reciprocal(out=rec, in_=den)
                    rec_b = bass.AP(tensor=rec.tensor, offset=rec.offset,
                                    ap=[rec.ap[0], rec.ap[1], [0, HID]])
                    hn = small.tile([P, HC], F32, tag="hn")
                    nc.vector.tensor_tensor(
                        out=hn.rearrange("p (h k) -> p h k", h=HEADS),
                        in0=ps[:, 0:HC].rearrange("p (h k) -> p h k", h=HEADS),
                        in1=rec_b, op=mybir.AluOpType.mult)
                    nc.vector.tensor_tensor(out=hn, in0=hn, in1=brow,
                                            op=mybir.AluOpType.add)
                    emin = small.tile([P, HC], F32, tag="emin")
                    nc.vector.tensor_scalar_min(emin, hn, 0.0)
                    eex = small.tile([P, HC], F32, tag="eex")
                    nc.scalar.activation(out=eex, in_=emin, func=EXPF)
                    nc.vector.tensor_scalar_max(hn, hn, 0.0)
                    nc.vector.tensor_tensor(out=hn, in0=hn, in1=eex,
                                            op=mybir.AluOpType.add)
                    hn16 = small.tile([P, HC], F16, tag="hn16")
                    nc.vector.tensor_scalar_add(hn16, hn, -1.0)
                    after_block(b, hn16)

        # ---------------- layer 1 dense + AllGather
        for b in range(NBLK):
            dense_block(0, b)
        nc.gpsimd.collective_compute(
            "AllGather", mybir.AluOpType.bypass, replica_groups=groups,
            ins=[tab_sh[0][:]], outs=[tab_full[0][:]])

        # ---------------- layer 1 edge (+ layer 2 dense interleaved)
        def after1(b, hn16):
            tp = pst.tile([P, P], F16, tag="tr")
            nc.tensor.transpose(out=tp, in_=hn16[:, 0:P], identity=ident16)
            nc.vector.tensor_copy(out=hT[:, 2 * b, :], in_=tp)
            tp2 = pst.tile([P, P], F16, tag="tr")
            nc.tensor.transpose(out=tp2, in_=hn16[:, P:HC], identity=ident16)
            nc.vector.tensor_copy(out=hT[:, 2 * b + 1, :], in_=tp2)
            dense_block(1, b)
        edge12(0, ws1, b1r, after1)
        nc.gpsimd.collective_compute(
            "AllGather", mybir.AluOpType.bypass, replica_groups=groups,
            ins=[tab_sh[1][:]], outs=[tab_full[1][:]])

        # ---------------- layer 2 edge (+ layer 3 dense inline)
        def after2(b, hn16):
            t3 = small.tile([P, HC], F16, tag="t3")
            nc.vector.tensor_tensor(out=t3, in0=hn16, in1=w3r,
                                    op=mybir.AluOpType.mult)
            h3 = small.tile([P, 1], F32, tag="h3")
            nc.vector.tensor_reduce(out=h3, in_=t3, axis=mybir.AxisListType.X,
                                    op=mybir.AluOpType.add)
            row3 = small.tile([P, 2], F16, tag="row3")
            nc.vector.tensor_copy(out=row3[:, 0:1], in_=h3)
            nc.vector.tensor_tensor(out=row3[:, 1:2], in0=h3, in1=sc3[:, 0:1],
                                    op=mybir.AluOpType.mult)
            nc.sync.dma_start(
                out=tab3_sh.rearrange("(b p) a -> p b a", p=P)[:, b, 0:2],
                in_=row3)
            ad3v = small.tile([P, 1], F16, tag="ad3v")
            nc.vector.tensor_tensor(out=ad3v, in0=h3, in1=sc3[:, 1:2],
                                    op=mybir.AluOpType.mult)
            nc.sync.dma_start(
                out=ad_t[2].rearrange("(b p) a -> p b a", p=P)[:, b, 0:1],
                in_=ad3v)
        edge12(1, ws2, b2r, after2)
        nc.gpsimd.collective_compute(
            "AllGather", mybir.AluOpType.bypass, replica_groups=groups,
            ins=[tab3_sh[:]], outs=[tab3_full[:]])

        # ---------------- layer 3 edge
        for c in range(NCHUNK):
            isl = slice(c * IW, (c + 1) * IW)
            g3 = apool.tile([P, CT, ADW], F16, tag="g3")
            nc.gpsimd.dma_gather(g3, tab3_full[:], srcg[:, isl],
                                 NIDX, nidx_reg, ADW, single_packet=False)
            d3 = apool.tile([P, CT, ADW], F16, tag="ad")
            nc.gpsimd.dma_gather(d3, ad_t[2][:], dstl[:, isl],
                                 NIDX, nidx_reg, ADW, single_packet=False)
            S = build_s(c)
            for bi in range(CBLK):
                b = c * CBLK + bi
                tsl = slice(bi * T, (bi + 1) * T)
                e3 = small.tile([P, T, 1], F32, tag="e3")
                nc.vector.tensor_tensor(out=e3, in0=g3[:, tsl, 1:2],
                                        in1=d3[:, tsl, 0:1],
                                        op=mybir.AluOpType.add)
                lk3 = small.tile([P, T, 1], F32, tag="lk3")
                nc.vector.tensor_scalar_mul(lk3, e3, NEG)
                nc.vector.tensor_tensor(out=lk3, in0=lk3, in1=e3,
                                        op=mybir.AluOpType.max)
                ex3 = small.tile([P, T, 1], F32, tag="ex3")
                nc.scalar.activation(out=ex3, in_=lk3, func=EXPF)
                m3 = small.tile([P, T, 2], F16, tag="m3")
                nc.vector.tensor_tensor(out=m3[:, :, 0:1], in0=ex3,
                                        in1=g3[:, tsl, 0:1],
                                        op=mybir.AluOpType.mult)
                nc.vector.tensor_copy(out=m3[:, :, 1:2], in_=ex3)
                ps3f = pse.tile([P, HC + HEADS], F32, tag="agg")
                ps3 = ps3f[:, 0:2]
                for t in range(T):
                    nc.tensor.matmul(ps3, S[:, bi * T + t, :], m3[:, t, :],
                                     start=(t == 0), stop=(t == T - 1))
                den3 = small.tile([P, 1], F32, tag="den3")
                nc.vector.tensor_scalar_max(den3, ps3[:, 1:2], 1e-30)
                rec3 = small.tile([P, 1], F32, tag="rec3")
                nc.vector.reciprocal(out=rec3, in_=den3)
                nc.vector.tensor_tensor(out=outsb[:, b:b + 1], in0=ps3[:, 0:1],
                                        in1=rec3, op=mybir.AluOpType.mult)
        # + b3
        nc.vector.tensor_tensor(out=outsb, in0=outsb,
                                in1=bass.AP(tensor=sc3.tensor,
                                            offset=sc3[:, 2:3].offset,
                                            ap=[list(sc3.ap[0]), [0, NBLK]]),
                                op=mybir.AluOpType.add)
        nc.sync.dma_start(out=out_p[:], in_=outsb)

    mybir.codegen_inst_isa_subclasses(nc)
    return nc


_CACHE = {}


def kernel(x, edge_index, W1, a_src1, a_dst1, b1, W2, a_src2, a_dst2, b2,
           W3, a_src3, a_dst3, b3):
    T, gslot, node_of_slot, srcg_idx, dstl_idx, dblk = _preprocess(
        np.asarray(edge_index))

    wa1 = _aug_weights(np.asarray(W1, np.float32), np.asarray(a_dst1, np.float32),
                       HEADS, HID)
    wa2 = _aug_weights(np.asarray(W2, np.float32), np.asarray(a_dst2, np.float32),
                       HEADS, HID)
    ws1 = np.asarray(a_src1, np.float32).reshape(1, HC).astype(np.float16)
    ws2 = np.asarray(a_src2, np.float32).reshape(1, HC).astype(np.float16)
    w3 = np.asarray(W3, np.float32).reshape(1, HC).astype(np.float16)
    sc3 = np.array([[float(np.asarray(a_src3).reshape(-1)[0]),
                     float(np.asarray(a_dst3).reshape(-1)[0]),
                     float(np.asarray(b3).reshape(-1)[0]), 0.0]], np.float32)
    iota = np.arange(P, dtype=np.float16).reshape(1, P)
    b1r = np.asarray(b1, np.float32).reshape(1, HC)
    b2r = np.asarray(b2, np.float32).reshape(1, HC)

    x = np.asarray(x, np.float32)
    in_maps = []
    for c in range(NCORES):
        sl = slice(c * SLOTS, (c + 1) * SLOTS)
        nos = node_of_slot[sl]
        xs = np.zeros((SLOTS, IN_DIM), np.float32)
        valid = nos >= 0
        xs[valid] = x[nos[valid]]
        xT = xs.T.astype(np.float16).reshape(P, SLOTS)
        in_maps.append({
            "xT_in": xT,
            "srcg_in": srcg_idx[c], "dstl_in": dstl_idx[c], "dblk_in": dblk[c],
            "wa1_in": wa1, "wa2_in": wa2, "ws1_in": ws1, "ws2_in": ws2,
            "w3_in": w3, "sc3_in": sc3, "b1_in": b1r, "b2_in": b2r,
            "iota_in": iota,
            "ident_in": np.eye(P, dtype=np.float16),
        })

    if T not in _CACHE:
        _CACHE[T] = _build(T)
    nc = _CACHE[T]
    res = run_bass_kernel_spmd(nc, in_maps, list(range(NCORES)))

    out = np.empty(N_NODES, np.float32)
    for c in range(NCORES):
        o = res.results[c]["out_p"]          # [P, NBLK]
        flat = o.T.reshape(-1)               # slot-major: b*P + p
        nos = node_of_slot[c * SLOTS:(c + 1) * SLOTS]
        valid = nos >= 0
        out[nos[valid]] = flat[valid]
    return out


# revision 3
# speedup vs baseline: 1.2710x; 1.2710x over previous
"""3-layer GAT (PyG GATConv semantics) on 8 Trainium2 NeuronCores — v3.

Strategy (dst-sharded, per-tile indirect gathers, lean compute):
- Nodes assigned to 160 degree-balanced blocks of <=128 dst slots; 20 blocks/core.
- Per layer l in {1,2}: dense phase (f16 matmuls) computes [h | alpha_src | alpha_dst]
  per block; h|alpha_src (260 f16 = 520B rows) go to the AllGather'd table,
  alpha_dst stays in SBUF per dst block.
- Edge phase per 128-edge tile: one [128,1] indirect-DMA gather of table rows by edge
  src (the only per-edge data movement; InstDMACopy dynamic path coexists with
  full-speed DVE, unlike InstDMAGatherAnt which starves it). Per-edge alpha_dst via
  PE: transpose the S selector tile and matmul against the block's alpha_dst column,
  accumulated into one per-block [128, T*4] PSUM. e=leaky(as+ad), ex=exp(e) (f32->f16),
  m = ex*h; aggregation + softmax denominators via one PE matmul per tile
  (lhsT = S 0/1 selector from iota-compare, rhs = [m | ex]).
- Layer 3 (heads=1, C=1): 8B f32 rows [h3 | a_src3*h3], same scheme.
- Dense phases run interleaved with the previous layer's edge blocks.

The walrus in this toolchain accepts only ONE sync wait per instruction; BassOneWait
splits Tile-generated multi-waits into single-wait EventSemaphore ops at serialization.
"""
import numpy as np
from contextlib import ExitStack
import heapq

import orjson
import concourse.bass as bass
import concourse.tile as tile
from concourse import mybir
from concourse.bass_utils import run_bass_kernel_spmd

# problem constants (fixed by the harness's setup_inputs)
N_NODES = 20000
N_EDGES = 320000
IN_DIM = 128
HID = 64
HEADS = 4
HC = HEADS * HID          # 256
ROWW = HC + HEADS         # 260 = table row: [h | alpha_src]
WAUG = HC + 2 * HEADS     # 264 = dense out: [h | alpha_src | alpha_dst]
NEG = 0.2
NCORES = 8
P = 128
NBLK = 20                 # dst blocks per core
SLOTS = NBLK * P          # 2560 slots per core
TOT_SLOTS = SLOTS * NCORES
CBLK = 2                  # blocks per S-chunk
NCHUNK = NBLK // CBLK

F32 = mybir.dt.float32
F16 = mybir.dt.float16
I32 = mybir.dt.int32

EXPF = mybir.ActivationFunctionType.Exp


def _split_multiwaits(bir: bytes) -> bytes:
    """Walrus here allows only 1 sync wait per instruction -> hoist extras onto
    same-engine EventSemaphore waits (dedup repeated ge-waits per engine; sems
    are monotonic within the block, so a repeated >= wait is a no-op)."""
    j = orjson.loads(bir)
    ctr = 0
    for fn in j["functions"]:
        for blk in fn["blocks"]:
            out_l = []
            last_wait = {}
            for ins in blk["instructions"]:
                eng = ins.get("engine")
                si = ins.get("sync_info")
                ow = (si or {}).get("on_wait") or []
                keep = 1
                if len(ow) > keep:
                    seen = last_wait.setdefault(eng, set())
                    for w in ow[:len(ow) - keep]:
                        key = (w.get("id"), w.get("wait_mode"), w.get("wait_value"))
                        if w.get("wait_mode") == "sem-ge-imm":
                            if key in seen:
                                continue
                            seen.add(key)
                        ctr += 1
                        out_l.append({
                            "engine": eng, "ins": [], "outs": [],
                            "name": f"mwsplit-{ctr}", "opcode": "EventSemaphore",
                            "sync_info": {"on_update": [], "on_wait": [w]},
                        })
                    si["on_wait"] = ow[len(ow) - keep:]
                out_l.append(ins)
            blk["instructions"] = out_l
    return orjson.dumps(j)


class BassOneWait(bass.Bass):
    def to_json_bytes(self):
        return _split_multiwaits(super().to_json_bytes())


# ---------------------------------------------------------------- host prep

def _preprocess(edge_index):
    """Assign nodes to degree-balanced blocks; build per-core edge arrays."""
    src = np.asarray(edge_index[0], dtype=np.int64)
    dst = np.asarray(edge_index[1], dtype=np.int64)
    loops = np.arange(N_NODES, dtype=np.int64)
    src = np.concatenate([src, loops])
    dst = np.concatenate([dst, loops])
    deg = np.bincount(dst, minlength=N_NODES).astype(np.int64)

    NB_TOT = NCORES * NBLK
    order = np.argsort(-deg, kind="stable")
    blk_of = np.empty(N_NODES, np.int32)
    slot_of = np.empty(N_NODES, np.int32)
    heap = [(0, 0, b) for b in range(NB_TOT)]
    heapq.heapify(heap)
    cnt = np.zeros(NB_TOT, np.int32)
    load = np.zeros(NB_TOT, np.int64)
    for n in order:
        while True:
            l, _, b = heapq.heappop(heap)
            if cnt[b] < P:
                break
        blk_of[n] = b
        slot_of[n] = cnt[b]
        cnt[b] += 1
        load[b] += deg[n]
        if cnt[b] < P:
            heapq.heappush(heap, (load[b], cnt[b], b))

    T = int(np.ceil(load.max() / P))
    gslot = blk_of.astype(np.int64) * P + slot_of
    node_of_slot = np.full(NB_TOT * P, -1, np.int64)
    node_of_slot[gslot] = np.arange(N_NODES)

    # bucket edges by dst block, sorted by src gslot for HBM locality
    eb = blk_of[dst]
    order_e = np.lexsort((gslot[src], eb))
    src_s = src[order_e]
    dst_s = dst[order_e]
    eb_s = eb[order_e]
    starts = np.searchsorted(eb_s, np.arange(NB_TOT + 1))

    NT = NBLK * T
    srcg = np.zeros((NCORES, P, NT), np.int32)
    dblk = np.full((NCORES, P, NT), -1.0, np.float16)
    for b in range(NB_TOT):
        c, lb = divmod(b, NBLK)
        e0, e1 = starts[b], starts[b + 1]
        k = e1 - e0
        col = np.zeros(T * P, np.int64)
        col[:k] = gslot[src_s[e0:e1]]
        srcg[c, :, lb * T:(lb + 1) * T] = col.reshape(T, P).T
        col_b = np.full(T * P, -1.0, np.float32)
        col_b[:k] = slot_of[dst_s[e0:e1]]
        dblk[c, :, lb * T:(lb + 1) * T] = col_b.reshape(T, P).T.astype(np.float16)

    return T, gslot, node_of_slot, srcg, dblk


def _aug_weights(W, a_src, a_dst, heads, hid):
    """[W | ws | wd], f16: ws[:,h] = W[:,h*hid:(h+1)*hid] @ a_src[h]."""
    cin = W.shape[0]
    ws = np.zeros((cin, heads), np.float32)
    wd = np.zeros((cin, heads), np.float32)
    for h in range(heads):
        blk = W[:, h * hid:(h + 1) * hid]
        ws[:, h] = blk @ a_src[h]
        wd[:, h] = blk @ a_dst[h]
    return np.concatenate([W, ws, wd], axis=1).astype(np.float16)


# ---------------------------------------------------------------- device kernel

def _build(T):
    NT = NBLK * T
    CT = CBLK * T
    nc = BassOneWait()
    dp = nc.declare_dram_parameter
    xT_in = dp("xT_in", [P, NBLK * P], F16, isOutput=False)
    srcg_in = dp("srcg_in", [P, NT], I32, isOutput=False)
    dblk_in = dp("dblk_in", [P, NT], F16, isOutput=False)
    wa1_in = dp("wa1_in", [IN_DIM, WAUG], F16, isOutput=False)
    wa2_in = dp("wa2_in", [HC, WAUG], F16, isOutput=False)
    w3_in = dp("w3_in", [1, HC], F16, isOutput=False)
    sc3_in = dp("sc3_in", [1, 4], F32, isOutput=False)
    b1_in = dp("b1_in", [1, HC], F32, isOutput=False)
    b2_in = dp("b2_in", [1, HC], F32, isOutput=False)
    iota_in = dp("iota_in", [1, P], F16, isOutput=False)
    ident_in = dp("ident_in", [P, P], F16, isOutput=False)
    out_p = dp("out_p", [P, NBLK], F32, isOutput=True)

    tab_sh = [nc.dram_tensor(f"tab_sh{l}", [SLOTS, ROWW], F16) for l in (1, 2)]
    tab_full = [nc.dram_tensor(f"tab_full{l}", [TOT_SLOTS, ROWW], F16) for l in (1, 2)]
    tab3_sh = nc.dram_tensor("tab3_sh", [SLOTS, 2], F32)
    tab3_full = nc.dram_tensor("tab3_full", [TOT_SLOTS, 2], F32)

    groups = [list(range(NCORES))]

    with tile.TileContext(nc) as tc, ExitStack() as ctx:
        consts = ctx.enter_context(tc.tile_pool(name="consts", bufs=1))
        meta = ctx.enter_context(tc.tile_pool(name="meta", bufs=1))
        spool = ctx.enter_context(tc.tile_pool(name="spool", bufs=2))
        gpool = ctx.enter_context(tc.tile_pool(name="gpool", bufs=4))
        mpool = ctx.enter_context(tc.tile_pool(name="mpool", bufs=2))
        small = ctx.enter_context(tc.tile_pool(name="small", bufs=2))
        sttp = ctx.enter_context(tc.tile_pool(name="sttp", bufs=3))
        psd = ctx.enter_context(tc.tile_pool(name="psd", bufs=2, space="PSUM"))
        pse = ctx.enter_context(tc.tile_pool(name="pse", bufs=2, space="PSUM"))
        pst = ctx.enter_context(tc.tile_pool(name="pst", bufs=2, space="PSUM"))
        psa = ctx.enter_context(tc.tile_pool(name="psa", bufs=2, space="PSUM"))

        # ---- constants / metadata
        ident16 = consts.tile([P, P], F16)
        nc.sync.dma_start(out=ident16, in_=ident_in[:])
        wa1 = consts.tile([P, WAUG], F16)
        nc.sync.dma_start(out=wa1, in_=wa1_in[:])
        wa2 = consts.tile([P, 2, WAUG], F16)
        nc.sync.dma_start(out=wa2, in_=wa2_in.rearrange("(j p) a -> p j a", p=P))

        def rep_load(name, src, n, dt):
            t = consts.tile([P, n], dt, tag=name)
            bc = bass.AP(tensor=src.tensor, offset=0, ap=[[0, P], [1, n]])
            nc.sync.dma_start(out=t, in_=bc)
            return t
        w3r = rep_load("w3r", w3_in[:], HC, F16)
        sc3 = rep_load("sc3", sc3_in[:], 4, F32)
        b1r = rep_load("b1r", b1_in[:], HC, F32)
        b2r = rep_load("b2r", b2_in[:], HC, F32)
        iot = rep_load("iot", iota_in[:], P, F16)

        dblk = meta.tile([P, NT], F16)
        nc.sync.dma_start(out=dblk, in_=dblk_in[:])
        srcg = meta.tile([P, NT], I32)
        nc.sync.dma_start(out=srcg, in_=srcg_in[:])
        hT = meta.tile([P, 2 * NBLK, P], F16)
        nc.sync.dma_start(out=hT[:, 0:NBLK, :],
                          in_=xT_in.rearrange("p (b n) -> p b n", n=P))
        outsb = meta.tile([P, NBLK], F32)
        adl = meta.tile([P, NBLK, HEADS], F16, tag="adl")    # layer 1/2 alpha_dst
        adl3 = meta.tile([P, NBLK, 1], F16, tag="adl3")

        def bcast_row(t, shape):
            ap = [list(t.ap[0])]
            for s in shape[1:-1]:
                ap.append([0, s])
            ap.append([t.ap[-1][0], shape[-1]])
            return bass.AP(tensor=t.tensor, offset=t.offset, ap=ap)

        def dense_block(lidx, b):
            ps = psd.tile([P, WAUG], F32, tag="dense")
            if lidx == 0:
                nc.tensor.matmul(ps, hT[:, b, :], wa1, start=True, stop=True)
            else:
                nc.tensor.matmul(ps, hT[:, 2 * b, :], wa2[:, 0, :],
                                 start=True, stop=False)
                nc.tensor.matmul(ps, hT[:, 2 * b + 1, :], wa2[:, 1, :],
                                 start=False, stop=True)
            tabt = small.tile([P, ROWW], F16, tag="tabt")
            nc.vector.tensor_copy(out=tabt, in_=ps[:, 0:ROWW])
            nc.sync.dma_start(
                out=tab_sh[lidx].rearrange("(b p) a -> p b a", p=P)[:, b, :],
                in_=tabt)
            nc.vector.tensor_copy(out=adl[:, b, :], in_=ps[:, ROWW:WAUG])

        def build_s(c):
            S = spool.tile([P, CT, P], F16, tag="S")
            sl = slice(c * CT, (c + 1) * CT)
            db_b = bass.AP(tensor=dblk.tensor, offset=dblk[:, sl].offset,
                           ap=[dblk.ap[0], [dblk.ap[1][0], CT], [0, P]])
            nc.vector.tensor_tensor(out=S, in0=db_b,
                                    in1=bcast_row(iot, [P, CT, P]),
                                    op=mybir.AluOpType.is_equal)
            return S

        def edge12(lidx, brow, after_block):
            for c in range(NCHUNK):
                S = build_s(c)
                for bi in range(CBLK):
                    b = c * CBLK + bi
                    hg = gpool.tile([P, T, ROWW], F16, tag="hg")
                    for t in range(T):
                        gt = b * T + t
                        nc.gpsimd.indirect_dma_start(
                            out=hg[:, t, :], out_offset=None,
                            in_=tab_full[lidx][:],
                            in_offset=bass.IndirectOffsetOnAxis(
                                ap=srcg[:, gt:gt + 1], axis=0))
                    # per-edge alpha_dst: transpose S_t, matmul vs block col
                    adx = psa.tile([P, T, HEADS], F32, tag="adx")
                    for t in range(T):
                        stp = pst.tile([P, P], F16, tag="tr")
                        nc.tensor.transpose(out=stp, in_=S[:, bi * T + t, :],
                                            identity=ident16)
                        stt = sttp.tile([P, P], F16, tag="stt")
                        nc.vector.tensor_copy(out=stt, in_=stp)
                        nc.tensor.matmul(adx[:, t, :], stt, adl[:, b, :],
                                         start=True, stop=True)
                    asum = small.tile([P, T, HEADS], F32, tag="asum")
                    nc.vector.tensor_tensor(out=asum, in0=adx,
                                            in1=hg[:, :, HC:ROWW],
                                            op=mybir.AluOpType.add)
                    lk = small.tile([P, T, HEADS], F32, tag="lk")
                    nc.vector.tensor_scalar_mul(lk, asum, NEG)
                    nc.vector.tensor_tensor(out=lk, in0=lk, in1=asum,
                                            op=mybir.AluOpType.max)
                    exf = small.tile([P, T, HEADS], F16, tag="exf")
                    nc.scalar.activation(out=exf, in_=lk, func=EXPF)
                    m = mpool.tile([P, T, ROWW], F16, tag="m")
                    ex_b = bass.AP(tensor=exf.tensor, offset=exf.offset,
                                   ap=[exf.ap[0], exf.ap[1], exf.ap[2], [0, HID]])
                    nc.vector.tensor_tensor(
                        out=m[:, :, 0:HC].rearrange("p t (h k) -> p t h k", h=HEADS),
                        in0=hg[:, :, 0:HC].rearrange("p t (h k) -> p t h k", h=HEADS),
                        in1=ex_b, op=mybir.AluOpType.mult)
                    nc.vector.tensor_copy(out=m[:, :, HC:ROWW], in_=exf)

                    ps = pse.tile([P, ROWW], F32, tag="agg")
                    for t in range(T):
                        nc.tensor.matmul(ps, S[:, bi * T + t, :], m[:, t, :],
                                         start=(t == 0), stop=(t == T - 1))

                    den = small.tile([P, HEADS], F32, tag="den")
                    nc.vector.tensor_scalar_max(den, ps[:, HC:ROWW], 1e-30)
                    rec = small.tile([P, HEADS], F32, tag="rec")
                    nc.vector.reciprocal(out=rec, in_=den)
                    rec_b = bass.AP(tensor=rec.tensor, offset=rec.offset,
                                    ap=[rec.ap[0], rec.ap[1], [0, HID]])
                    hn = small.tile([P, HC], F32, tag="hn")
                    nc.vector.tensor_tensor(
                        out=hn.rearrange("p (h k) -> p h k", h=HEADS),
                        in0=ps[:, 0:HC].rearrange("p (h k) -> p h k", h=HEADS),
                        in1=rec_b, op=mybir.AluOpType.mult)
                    nc.vector.tensor_tensor(out=hn, in0=hn, in1=brow,
                                            op=mybir.AluOpType.add)
                    emin = small.tile([P, HC], F32, tag="emin")
                    nc.vector.tensor_scalar_min(emin, hn, 0.0)
                    eex = small.tile([P, HC], F32, tag="eex")
                    nc.scalar.activation(out=eex, in_=emin, func=EXPF)
                    nc.vector.tensor_scalar_max(hn, hn, 0.0)
                    nc.vector.tensor_tensor(out=hn, in0=hn, in1=eex,
                                            op=mybir.AluOpType.add)
                    hn16 = small.tile([P, HC], F16, tag="hn16")
                    nc.vector.tensor_scalar_add(hn16, hn, -1.0)
                    after_block(b, hn16)

        # ---------------- layer 1 dense + AllGather
        for b in range(NBLK):
            dense_block(0, b)
        nc.gpsimd.collective_compute(
            "AllGather", mybir.AluOpType.bypass, replica_groups=groups,
            ins=[tab_sh[0][:]], outs=[tab_full[0][:]])

        # ---------------- layer 1 edge (+ layer 2 dense interleaved)
        def after1(b, hn16):
            tp = pst.tile([P, P], F16, tag="tr")
            nc.tensor.transpose(out=tp, in_=hn16[:, 0:P], identity=ident16)
            nc.vector.tensor_copy(out=hT[:, 2 * b, :], in_=tp)
            tp2 = pst.tile([P, P], F16, tag="tr")
            nc.tensor.transpose(out=tp2, in_=hn16[:, P:HC], identity=ident16)
            nc.vector.tensor_copy(out=hT[:, 2 * b + 1, :], in_=tp2)
            dense_block(1, b)
        edge12(0, b1r, after1)
        nc.gpsimd.collective_compute(
            "AllGather", mybir.AluOpType.bypass, replica_groups=groups,
            ins=[tab_sh[1][:]], outs=[tab_full[1][:]])

        # ---------------- layer 2 edge (+ layer 3 dense inline)
        def after2(b, hn16):
            t3 = small.tile([P, HC], F16, tag="t3")
            nc.vector.tensor_tensor(out=t3, in0=hn16, in1=w3r,
                                    op=mybir.AluOpType.mult)
            h3 = small.tile([P, 1], F32, tag="h3")
            nc.vector.tensor_reduce(out=h3, in_=t3, axis=mybir.AxisListType.X,
                                    op=mybir.AluOpType.add)
            row3 = small.tile([P, 2], F32, tag="row3")
            nc.vector.tensor_copy(out=row3[:, 0:1], in_=h3)
            nc.vector.tensor_tensor(out=row3[:, 1:2], in0=h3, in1=sc3[:, 0:1],
                                    op=mybir.AluOpType.mult)
            nc.sync.dma_start(
                out=tab3_sh.rearrange("(b p) a -> p b a", p=P)[:, b, :],
                in_=row3)
            nc.vector.tensor_tensor(out=adl3[:, b, :], in0=h3, in1=sc3[:, 1:2],
                                    op=mybir.AluOpType.mult)
        edge12(1, b2r, after2)
        nc.gpsimd.collective_compute(
            "AllGather", mybir.AluOpType.bypass, replica_groups=groups,
            ins=[tab3_sh[:]], outs=[tab3_full[:]])

        # ---------------- layer 3 edge
        for c in range(NCHUNK):
            S = build_s(c)
            for bi in range(CBLK):
                b = c * CBLK + bi
                g3 = gpool.tile([P, T, 2], F32, tag="g3")
                for t in range(T):
                    gt = b * T + t
                    nc.gpsimd.indirect_dma_start(
                        out=g3[:, t, :], out_offset=None, in_=tab3_full[:],
                        in_offset=bass.IndirectOffsetOnAxis(
                            ap=srcg[:, gt:gt + 1], axis=0))
                d3 = psa.tile([P, T, HEADS], F32, tag="adx")
                for t in range(T):
                    stp = pst.tile([P, P], F16, tag="tr")
                    nc.tensor.transpose(out=stp, in_=S[:, bi * T + t, :],
                                        identity=ident16)
                    stt = sttp.tile([P, P], F16, tag="stt")
                    nc.vector.tensor_copy(out=stt, in_=stp)
                    nc.tensor.matmul(d3[:, t, 0:1], stt, adl3[:, b, :],
                                     start=True, stop=True)
                e3 = small.tile([P, T, 1], F32, tag="e3")
                nc.vector.tensor_tensor(out=e3, in0=g3[:, :, 1:2],
                                        in1=d3[:, :, 0:1],
                                        op=mybir.AluOpType.add)
                lk3 = small.tile([P, T, 1], F32, tag="lk3")
                nc.vector.tensor_scalar_mul(lk3, e3, NEG)
                nc.vector.tensor_tensor(out=lk3, in0=lk3, in1=e3,
                                        op=mybir.AluOpType.max)
                ex3 = small.tile([P, T, 1], F32, tag="ex3")
                nc.scalar.activation(out=ex3, in_=lk3, func=EXPF)
                m3 = small.tile([P, T, 2], F16, tag="m3")
                nc.vector.tensor_tensor(out=m3[:, :, 0:1], in0=ex3,
                                        in1=g3[:, :, 0:1],
                                        op=mybir.AluOpType.mult)
                nc.vector.tensor_copy(out=m3[:, :, 1:2], in_=ex3)
                ps3f = pse.tile([P, ROWW], F32, tag="agg")
                ps3 = ps3f[:, 0:2]
                for t in range(T):
                    nc.tensor.matmul(ps3, S[:, bi * T + t, :], m3[:, t, :],
                                     start=(t == 0), stop=(t == T - 1))
                den3 = small.tile([P, 1], F32, tag="den3")
                nc.vector.tensor_scalar_max(den3, ps3[:, 1:2], 1e-30)
                rec3 = small.tile([P, 1], F32, tag="rec3")
                nc.vector.reciprocal(out=rec3, in_=den3)
                nc.vector.tensor_tensor(out=outsb[:, b:b + 1], in0=ps3[:, 0:1],
                                        in1=rec3, op=mybir.AluOpType.mult)
        nc.vector.tensor_tensor(out=outsb, in0=outsb,
                                in1=bass.AP(tensor=sc3.tensor,
                                            offset=sc3[:, 2:3].offset,
                                            ap=[list(sc3.ap[0]), [0, NBLK]]),
                                op=mybir.AluOpType.add)
        nc.sync.dma_start(out=out_p[:], in_=outsb)

    return nc


_CACHE = {}


def kernel(x, edge_index, W1, a_src1, a_dst1, b1, W2, a_src2, a_dst2, b2,
           W3, a_src3, a_dst3, b3):
    T, gslot, node_of_slot, srcg, dblk = _preprocess(np.asarray(edge_index))

    wa1 = _aug_weights(np.asarray(W1, np.float32), np.asarray(a_src1, np.float32),
                       np.asarray(a_dst1, np.float32), HEADS, HID)
    wa2 = _aug_weights(np.asarray(W2, np.float32), np.asarray(a_src2, np.float32),
                       np.asarray(a_dst2, np.float32), HEADS, HID)
    w3 = np.asarray(W3, np.float32).reshape(1, HC).astype(np.float16)
    sc3 = np.array([[float(np.asarray(a_src3).reshape(-1)[0]),
                     float(np.asarray(a_dst3).reshape(-1)[0]),
                     float(np.asarray(b3).reshape(-1)[0]), 0.0]], np.float32)
    iota = np.arange(P, dtype=np.float16).reshape(1, P)
    b1r = np.asarray(b1, np.float32).reshape(1, HC)
    b2r = np.asarray(b2, np.float32).reshape(1, HC)

    x = np.asarray(x, np.float32)
    in_maps = []
    for c in range(NCORES):
        sl = slice(c * SLOTS, (c + 1) * SLOTS)
        nos = node_of_slot[sl]
        xs = np.zeros((SLOTS, IN_DIM), np.float32)
        valid = nos >= 0
        xs[valid] = x[nos[valid]]
        xT = xs.T.astype(np.float16).reshape(P, SLOTS)
        in_maps.append({
            "xT_in": xT,
            "srcg_in": srcg[c], "dblk_in": dblk[c],
            "wa1_in": wa1, "wa2_in": wa2, "w3_in": w3, "sc3_in": sc3,
            "b1_in": b1r, "b2_in": b2r, "iota_in": iota,
            "ident_in": np.eye(P, dtype=np.float16),
        })

    if T not in _CACHE:
        _CACHE[T] = _build(T)
    nc = _CACHE[T]
    res = run_bass_kernel_spmd(nc, in_maps, list(range(NCORES)))

    out = np.empty(N_NODES, np.float32)
    for c in range(NCORES):
        o = res.results[c]["out_p"]
        flat = o.T.reshape(-1)
        nos = node_of_slot[c * SLOTS:(c + 1) * SLOTS]
        valid = nos >= 0
        out[nos[valid]] = flat[valid]
    return out


# revision 4
# speedup vs baseline: 1.3227x; 1.0407x over previous
"""3-layer GAT (PyG GATConv semantics) on 8 Trainium2 NeuronCores — v3.

Strategy (dst-sharded, per-tile indirect gathers, lean compute):
- Nodes assigned to 160 degree-balanced blocks of <=128 dst slots; 20 blocks/core.
- Per layer l in {1,2}: dense phase (f16 matmuls) computes [h | alpha_src | alpha_dst]
  per block; h|alpha_src (260 f16 = 520B rows) go to the AllGather'd table,
  alpha_dst stays in SBUF per dst block.
- Edge phase per 128-edge tile: one [128,1] indirect-DMA gather of table rows by edge
  src (the only per-edge data movement; InstDMACopy dynamic path coexists with
  full-speed DVE, unlike InstDMAGatherAnt which starves it). The 0/1 selector S
  (edge->dst-slot) and its transpose St are HOST-precomputed and streamed from DRAM
  (HWDGE), so the only per-tile PE work is: adx matmul (St @ alpha_dst column, into a
  per-block [128, T*4] PSUM) and the aggregation matmul (lhsT = S, rhs = [m | ex]).
  e=leaky(as+ad), ex=exp(e) (f32->f16), m = ex*h.
- Layer 3 (heads=1, C=1): 8B f32 rows [h3 | a_src3*h3], same scheme.
- Dense phases run interleaved with the previous layer's edge blocks.

The walrus in this toolchain accepts only ONE sync wait per instruction; BassOneWait
splits Tile-generated multi-waits into single-wait EventSemaphore ops at serialization.
"""
import numpy as np
from contextlib import ExitStack
import heapq

import orjson
import concourse.bass as bass
import concourse.tile as tile
from concourse import mybir
from concourse.bass_utils import run_bass_kernel_spmd

# problem constants (fixed by the harness's setup_inputs)
N_NODES = 20000
N_EDGES = 320000
IN_DIM = 128
HID = 64
HEADS = 4
HC = HEADS * HID          # 256
ROWW = HC + HEADS         # 260 = table row: [h | alpha_src]
WAUG = HC + 2 * HEADS     # 264 = dense out: [h | alpha_src | alpha_dst]
NEG = 0.2
NCORES = 8
P = 128
NBLK = 20                 # dst blocks per core
SLOTS = NBLK * P          # 2560 slots per core
TOT_SLOTS = SLOTS * NCORES
CBLK = 2                  # blocks per S-chunk
NCHUNK = NBLK // CBLK

F32 = mybir.dt.float32
F16 = mybir.dt.float16
I32 = mybir.dt.int32

EXPF = mybir.ActivationFunctionType.Exp


def _split_multiwaits(bir: bytes) -> bytes:
    """Walrus here allows only 1 sync wait per instruction -> hoist extras onto
    same-engine EventSemaphore waits (dedup repeated ge-waits per engine; sems
    are monotonic within the block, so a repeated >= wait is a no-op)."""
    j = orjson.loads(bir)
    ctr = 0
    for fn in j["functions"]:
        for blk in fn["blocks"]:
            out_l = []
            last_wait = {}
            for ins in blk["instructions"]:
                eng = ins.get("engine")
                si = ins.get("sync_info")
                ow = (si or {}).get("on_wait") or []
                keep = 1
                if len(ow) > keep:
                    seen = last_wait.setdefault(eng, set())
                    for w in ow[:len(ow) - keep]:
                        key = (w.get("id"), w.get("wait_mode"), w.get("wait_value"))
                        if w.get("wait_mode") == "sem-ge-imm":
                            if key in seen:
                                continue
                            seen.add(key)
                        ctr += 1
                        out_l.append({
                            "engine": eng, "ins": [], "outs": [],
                            "name": f"mwsplit-{ctr}", "opcode": "EventSemaphore",
                            "sync_info": {"on_update": [], "on_wait": [w]},
                        })
                    si["on_wait"] = ow[len(ow) - keep:]
                out_l.append(ins)
            blk["instructions"] = out_l
    return orjson.dumps(j)


class BassOneWait(bass.Bass):
    def to_json_bytes(self):
        return _split_multiwaits(super().to_json_bytes())


# ---------------------------------------------------------------- host prep

def _preprocess(edge_index):
    """Assign nodes to degree-balanced blocks; build per-core edge arrays."""
    src = np.asarray(edge_index[0], dtype=np.int64)
    dst = np.asarray(edge_index[1], dtype=np.int64)
    loops = np.arange(N_NODES, dtype=np.int64)
    src = np.concatenate([src, loops])
    dst = np.concatenate([dst, loops])
    deg = np.bincount(dst, minlength=N_NODES).astype(np.int64)

    NB_TOT = NCORES * NBLK
    order = np.argsort(-deg, kind="stable")
    blk_of = np.empty(N_NODES, np.int32)
    slot_of = np.empty(N_NODES, np.int32)
    heap = [(0, 0, b) for b in range(NB_TOT)]
    heapq.heapify(heap)
    cnt = np.zeros(NB_TOT, np.int32)
    load = np.zeros(NB_TOT, np.int64)
    for n in order:
        while True:
            l, _, b = heapq.heappop(heap)
            if cnt[b] < P:
                break
        blk_of[n] = b
        slot_of[n] = cnt[b]
        cnt[b] += 1
        load[b] += deg[n]
        if cnt[b] < P:
            heapq.heappush(heap, (load[b], cnt[b], b))

    T = int(np.ceil(load.max() / P))
    gslot = blk_of.astype(np.int64) * P + slot_of
    node_of_slot = np.full(NB_TOT * P, -1, np.int64)
    node_of_slot[gslot] = np.arange(N_NODES)

    # bucket edges by dst block, sorted by src gslot for HBM locality
    eb = blk_of[dst]
    order_e = np.lexsort((gslot[src], eb))
    src_s = src[order_e]
    dst_s = dst[order_e]
    eb_s = eb[order_e]
    starts = np.searchsorted(eb_s, np.arange(NB_TOT + 1))

    NT = NBLK * T
    srcg = np.zeros((NCORES, P, NT), np.int32)
    s_mat = np.zeros((NCORES, NT, P, P), np.float16)   # [tile, edge, slot]
    st_mat = np.zeros((NCORES, NT, P, P), np.float16)  # [tile, slot, edge]
    for b in range(NB_TOT):
        c, lb = divmod(b, NBLK)
        e0, e1 = starts[b], starts[b + 1]
        k = e1 - e0
        col = np.zeros(T * P, np.int64)
        col[:k] = gslot[src_s[e0:e1]]
        srcg[c, :, lb * T:(lb + 1) * T] = col.reshape(T, P).T
        sl = np.full(T * P, -1, np.int64)
        sl[:k] = slot_of[dst_s[e0:e1]]
        sl = sl.reshape(T, P)
        tt, ee = np.nonzero(sl >= 0)
        s_mat[c, lb * T + tt, ee, sl[tt, ee]] = 1.0
        st_mat[c, lb * T + tt, sl[tt, ee], ee] = 1.0

    return T, gslot, node_of_slot, srcg, s_mat, st_mat


def _aug_weights(W, a_src, a_dst, heads, hid):
    """[W | ws | wd], f16: ws[:,h] = W[:,h*hid:(h+1)*hid] @ a_src[h]."""
    cin = W.shape[0]
    ws = np.zeros((cin, heads), np.float32)
    wd = np.zeros((cin, heads), np.float32)
    for h in range(heads):
        blk = W[:, h * hid:(h + 1) * hid]
        ws[:, h] = blk @ a_src[h]
        wd[:, h] = blk @ a_dst[h]
    return np.concatenate([W, ws, wd], axis=1).astype(np.float16)


# ---------------------------------------------------------------- device kernel

def _build(T):
    NT = NBLK * T
    CT = CBLK * T
    nc = BassOneWait()
    dp = nc.declare_dram_parameter
    xT_in = dp("xT_in", [P, NBLK * P], F16, isOutput=False)
    srcg_in = dp("srcg_in", [P, NT], I32, isOutput=False)
    s_in = dp("s_in", [P, NT * P], F16, isOutput=False)
    st_in = dp("st_in", [P, NT * P], F16, isOutput=False)
    wa1_in = dp("wa1_in", [IN_DIM, WAUG], F16, isOutput=False)
    wa2_in = dp("wa2_in", [HC, WAUG], F16, isOutput=False)
    w3_in = dp("w3_in", [1, HC], F16, isOutput=False)
    sc3_in = dp("sc3_in", [1, 4], F32, isOutput=False)
    b1_in = dp("b1_in", [1, HC], F32, isOutput=False)
    b2_in = dp("b2_in", [1, HC], F32, isOutput=False)
    ident_in = dp("ident_in", [P, P], F16, isOutput=False)
    out_p = dp("out_p", [P, NBLK], F32, isOutput=True)

    tab_sh = [nc.dram_tensor(f"tab_sh{l}", [SLOTS, ROWW], F16) for l in (1, 2)]
    tab_full = [nc.dram_tensor(f"tab_full{l}", [TOT_SLOTS, ROWW], F16) for l in (1, 2)]
    tab3_sh = nc.dram_tensor("tab3_sh", [SLOTS, 2], F32)
    tab3_full = nc.dram_tensor("tab3_full", [TOT_SLOTS, 2], F32)

    groups = [list(range(NCORES))]

    with tile.TileContext(nc) as tc, ExitStack() as ctx:
        consts = ctx.enter_context(tc.tile_pool(name="consts", bufs=1))
        meta = ctx.enter_context(tc.tile_pool(name="meta", bufs=1))
        spool = ctx.enter_context(tc.tile_pool(name="spool", bufs=3))
        gpool = ctx.enter_context(tc.tile_pool(name="gpool", bufs=4))
        mpool = ctx.enter_context(tc.tile_pool(name="mpool", bufs=2))
        small = ctx.enter_context(tc.tile_pool(name="small", bufs=2))
        sttp = ctx.enter_context(tc.tile_pool(name="sttp", bufs=3))
        psd = ctx.enter_context(tc.tile_pool(name="psd", bufs=2, space="PSUM"))
        pse = ctx.enter_context(tc.tile_pool(name="pse", bufs=2, space="PSUM"))
        pst = ctx.enter_context(tc.tile_pool(name="pst", bufs=2, space="PSUM"))
        psa = ctx.enter_context(tc.tile_pool(name="psa", bufs=2, space="PSUM"))

        # ---- constants / metadata
        ident16 = consts.tile([P, P], F16)
        nc.sync.dma_start(out=ident16, in_=ident_in[:])
        wa1 = consts.tile([P, WAUG], F16)
        nc.sync.dma_start(out=wa1, in_=wa1_in[:])
        wa2 = consts.tile([P, 2, WAUG], F16)
        nc.sync.dma_start(out=wa2, in_=wa2_in.rearrange("(j p) a -> p j a", p=P))

        def rep_load(name, src, n, dt):
            t = consts.tile([P, n], dt, tag=name)
            bc = bass.AP(tensor=src.tensor, offset=0, ap=[[0, P], [1, n]])
            nc.sync.dma_start(out=t, in_=bc)
            return t
        w3r = rep_load("w3r", w3_in[:], HC, F16)
        sc3 = rep_load("sc3", sc3_in[:], 4, F32)
        b1r = rep_load("b1r", b1_in[:], HC, F32)
        b2r = rep_load("b2r", b2_in[:], HC, F32)

        srcg = meta.tile([P, NT], I32)
        nc.sync.dma_start(out=srcg, in_=srcg_in[:])
        hT = meta.tile([P, 2 * NBLK, P], F16)
        nc.sync.dma_start(out=hT[:, 0:NBLK, :],
                          in_=xT_in.rearrange("p (b n) -> p b n", n=P))
        outsb = meta.tile([P, NBLK], F32)
        adl = meta.tile([P, NBLK, HEADS], F16, tag="adl")    # layer 1/2 alpha_dst
        adl3 = meta.tile([P, NBLK, 1], F16, tag="adl3")

        def bcast_row(t, shape):
            ap = [list(t.ap[0])]
            for s in shape[1:-1]:
                ap.append([0, s])
            ap.append([t.ap[-1][0], shape[-1]])
            return bass.AP(tensor=t.tensor, offset=t.offset, ap=ap)

        def dense_block(lidx, b):
            ps = psd.tile([P, WAUG], F32, tag="dense")
            if lidx == 0:
                nc.tensor.matmul(ps, hT[:, b, :], wa1, start=True, stop=True)
            else:
                nc.tensor.matmul(ps, hT[:, 2 * b, :], wa2[:, 0, :],
                                 start=True, stop=False)
                nc.tensor.matmul(ps, hT[:, 2 * b + 1, :], wa2[:, 1, :],
                                 start=False, stop=True)
            tabt = small.tile([P, ROWW], F16, tag="tabt")
            nc.vector.tensor_copy(out=tabt, in_=ps[:, 0:ROWW])
            nc.sync.dma_start(
                out=tab_sh[lidx].rearrange("(b p) a -> p b a", p=P)[:, b, :],
                in_=tabt)
            nc.vector.tensor_copy(out=adl[:, b, :], in_=ps[:, ROWW:WAUG])

        s_view = s_in.rearrange("p (n q) -> p n q", q=P)    # [P, NT, P]: tile n -> [e, slot]
        st_view = st_in.rearrange("p (n q) -> p n q", q=P)

        def load_s(b):
            S = spool.tile([P, T, P], F16, tag="S")
            nc.sync.dma_start(out=S, in_=s_view[:, b * T:(b + 1) * T, :])
            St = sttp.tile([P, T, P], F16, tag="St")
            nc.sync.dma_start(out=St, in_=st_view[:, b * T:(b + 1) * T, :])
            return S, St

        def edge12(lidx, brow, after_block):
            for b in range(NBLK):
                    S, St = load_s(b)
                    hg = gpool.tile([P, T, ROWW], F16, tag="hg")
                    for t in range(T):
                        gt = b * T + t
                        nc.gpsimd.indirect_dma_start(
                            out=hg[:, t, :], out_offset=None,
                            in_=tab_full[lidx][:],
                            in_offset=bass.IndirectOffsetOnAxis(
                                ap=srcg[:, gt:gt + 1], axis=0))
                    # per-edge alpha_dst: St_t @ block's alpha_dst column
                    adx = psa.tile([P, T, HEADS], F32, tag="adx")
                    for t in range(T):
                        nc.tensor.matmul(adx[:, t, :], St[:, t, :], adl[:, b, :],
                                         start=True, stop=True)
                    asum = small.tile([P, T, HEADS], F32, tag="asum")
                    nc.vector.tensor_tensor(out=asum, in0=adx,
                                            in1=hg[:, :, HC:ROWW],
                                            op=mybir.AluOpType.add)
                    lk = small.tile([P, T, HEADS], F32, tag="lk")
                    nc.vector.tensor_scalar_mul(lk, asum, NEG)
                    nc.vector.tensor_tensor(out=lk, in0=lk, in1=asum,
                                            op=mybir.AluOpType.max)
                    exf = small.tile([P, T, HEADS], F16, tag="exf")
                    nc.scalar.activation(out=exf, in_=lk, func=EXPF)
                    m = mpool.tile([P, T, ROWW], F16, tag="m")
                    ex_b = bass.AP(tensor=exf.tensor, offset=exf.offset,
                                   ap=[exf.ap[0], exf.ap[1], exf.ap[2], [0, HID]])
                    nc.vector.tensor_tensor(
                        out=m[:, :, 0:HC].rearrange("p t (h k) -> p t h k", h=HEADS),
                        in0=hg[:, :, 0:HC].rearrange("p t (h k) -> p t h k", h=HEADS),
                        in1=ex_b, op=mybir.AluOpType.mult)
                    nc.vector.tensor_copy(out=m[:, :, HC:ROWW], in_=exf)

                    ps = pse.tile([P, ROWW], F32, tag="agg")
                    for t in range(T):
                        nc.tensor.matmul(ps, S[:, t, :], m[:, t, :],
                                         start=(t == 0), stop=(t == T - 1))

                    den = small.tile([P, HEADS], F32, tag="den")
                    nc.vector.tensor_scalar_max(den, ps[:, HC:ROWW], 1e-30)
                    rec = small.tile([P, HEADS], F32, tag="rec")
                    nc.vector.reciprocal(out=rec, in_=den)
                    rec_b = bass.AP(tensor=rec.tensor, offset=rec.offset,
                                    ap=[rec.ap[0], rec.ap[1], [0, HID]])
                    hn = small.tile([P, HC], F32, tag="hn")
                    nc.vector.tensor_tensor(
                        out=hn.rearrange("p (h k) -> p h k", h=HEADS),
                        in0=ps[:, 0:HC].rearrange("p (h k) -> p h k", h=HEADS),
                        in1=rec_b, op=mybir.AluOpType.mult)
                    nc.vector.tensor_tensor(out=hn, in0=hn, in1=brow,
                                            op=mybir.AluOpType.add)
                    emin = small.tile([P, HC], F32, tag="emin")
                    nc.vector.tensor_scalar_min(emin, hn, 0.0)
                    eex = small.tile([P, HC], F32, tag="eex")
                    nc.scalar.activation(out=eex, in_=emin, func=EXPF)
                    nc.vector.tensor_scalar_max(hn, hn, 0.0)
                    nc.vector.tensor_tensor(out=hn, in0=hn, in1=eex,
                                            op=mybir.AluOpType.add)
                    hn16 = small.tile([P, HC], F16, tag="hn16")
                    nc.vector.tensor_scalar_add(hn16, hn, -1.0)
                    after_block(b, hn16)

        # ---------------- layer 1 dense + AllGather
        for b in range(NBLK):
            dense_block(0, b)
        nc.gpsimd.collective_compute(
            "AllGather", mybir.AluOpType.bypass, replica_groups=groups,
            ins=[tab_sh[0][:]], outs=[tab_full[0][:]])

        # ---------------- layer 1 edge (+ layer 2 dense interleaved)
        def after1(b, hn16):
            tp = pst.tile([P, P], F16, tag="tr")
            nc.tensor.transpose(out=tp, in_=hn16[:, 0:P], identity=ident16)
            nc.vector.tensor_copy(out=hT[:, 2 * b, :], in_=tp)
            tp2 = pst.tile([P, P], F16, tag="tr")
            nc.tensor.transpose(out=tp2, in_=hn16[:, P:HC], identity=ident16)
            nc.vector.tensor_copy(out=hT[:, 2 * b + 1, :], in_=tp2)
            dense_block(1, b)
        edge12(0, b1r, after1)
        nc.gpsimd.collective_compute(
            "AllGather", mybir.AluOpType.bypass, replica_groups=groups,
            ins=[tab_sh[1][:]], outs=[tab_full[1][:]])

        # ---------------- layer 2 edge (+ layer 3 dense inline)
        def after2(b, hn16):
            t3 = small.tile([P, HC], F16, tag="t3")
            nc.vector.tensor_tensor(out=t3, in0=hn16, in1=w3r,
                                    op=mybir.AluOpType.mult)
            h3 = small.tile([P, 1], F32, tag="h3")
            nc.vector.tensor_reduce(out=h3, in_=t3, axis=mybir.AxisListType.X,
                                    op=mybir.AluOpType.add)
            row3 = small.tile([P, 2], F32, tag="row3")
            nc.vector.tensor_copy(out=row3[:, 0:1], in_=h3)
            nc.vector.tensor_tensor(out=row3[:, 1:2], in0=h3, in1=sc3[:, 0:1],
                                    op=mybir.AluOpType.mult)
            nc.sync.dma_start(
                out=tab3_sh.rearrange("(b p) a -> p b a", p=P)[:, b, :],
                in_=row3)
            nc.vector.tensor_tensor(out=adl3[:, b, :], in0=h3, in1=sc3[:, 1:2],
                                    op=mybir.AluOpType.mult)
        edge12(1, b2r, after2)
        nc.gpsimd.collective_compute(
            "AllGather", mybir.AluOpType.bypass, replica_groups=groups,
            ins=[tab3_sh[:]], outs=[tab3_full[:]])

        # ---------------- layer 3 edge
        for b in range(NBLK):
                S, St = load_s(b)
                g3 = gpool.tile([P, T, 2], F32, tag="g3")
                for t in range(T):
                    gt = b * T + t
                    nc.gpsimd.indirect_dma_start(
                        out=g3[:, t, :], out_offset=None, in_=tab3_full[:],
                        in_offset=bass.IndirectOffsetOnAxis(
                            ap=srcg[:, gt:gt + 1], axis=0))
                d3 = psa.tile([P, T, HEADS], F32, tag="adx")
                for t in range(T):
                    nc.tensor.matmul(d3[:, t, 0:1], St[:, t, :], adl3[:, b, :],
                                     start=True, stop=True)
                e3 = small.tile([P, T, 1], F32, tag="e3")
                nc.vector.tensor_tensor(out=e3, in0=g3[:, :, 1:2],
                                        in1=d3[:, :, 0:1],
                                        op=mybir.AluOpType.add)
                lk3 = small.tile([P, T, 1], F32, tag="lk3")
                nc.vector.tensor_scalar_mul(lk3, e3, NEG)
                nc.vector.tensor_tensor(out=lk3, in0=lk3, in1=e3,
                                        op=mybir.AluOpType.max)
                ex3 = small.tile([P, T, 1], F32, tag="ex3")
                nc.scalar.activation(out=ex3, in_=lk3, func=EXPF)
                m3 = small.tile([P, T, 2], F16, tag="m3")
                nc.vector.tensor_tensor(out=m3[:, :, 0:1], in0=ex3,
                                        in1=g3[:, :, 0:1],
                                        op=mybir.AluOpType.mult)
                nc.vector.tensor_copy(out=m3[:, :, 1:2], in_=ex3)
                ps3f = pse.tile([P, ROWW], F32, tag="agg")
                ps3 = ps3f[:, 0:2]
                for t in range(T):
                    nc.tensor.matmul(ps3, S[:, t, :], m3[:, t, :],
                                     start=(t == 0), stop=(t == T - 1))
                den3 = small.tile([P, 1], F32, tag="den3")
                nc.vector.tensor_scalar_max(den3, ps3[:, 1:2], 1e-30)
                rec3 = small.tile([P, 1], F32, tag="rec3")
                nc.vector.reciprocal(out=rec3, in_=den3)
                nc.vector.tensor_tensor(out=outsb[:, b:b + 1], in0=ps3[:, 0:1],
                                        in1=rec3, op=mybir.AluOpType.mult)
        nc.vector.tensor_tensor(out=outsb, in0=outsb,
                                in1=bass.AP(tensor=sc3.tensor,
                                            offset=sc3[:, 2:3].offset,
                                            ap=[list(sc3.ap[0]), [0, NBLK]]),
                                op=mybir.AluOpType.add)
        nc.sync.dma_start(out=out_p[:], in_=outsb)

    return nc


_CACHE = {}


def kernel(x, edge_index, W1, a_src1, a_dst1, b1, W2, a_src2, a_dst2, b2,
           W3, a_src3, a_dst3, b3):
    T, gslot, node_of_slot, srcg, s_mat, st_mat = _preprocess(np.asarray(edge_index))

    wa1 = _aug_weights(np.asarray(W1, np.float32), np.asarray(a_src1, np.float32),
                       np.asarray(a_dst1, np.float32), HEADS, HID)
    wa2 = _aug_weights(np.asarray(W2, np.float32), np.asarray(a_src2, np.float32),
                       np.asarray(a_dst2, np.float32), HEADS, HID)
    w3 = np.asarray(W3, np.float32).reshape(1, HC).astype(np.float16)
    sc3 = np.array([[float(np.asarray(a_src3).reshape(-1)[0]),
                     float(np.asarray(a_dst3).reshape(-1)[0]),
                     float(np.asarray(b3).reshape(-1)[0]), 0.0]], np.float32)
    b1r = np.asarray(b1, np.float32).reshape(1, HC)
    b2r = np.asarray(b2, np.float32).reshape(1, HC)

    x = np.asarray(x, np.float32)
    in_maps = []
    for c in range(NCORES):
        sl = slice(c * SLOTS, (c + 1) * SLOTS)
        nos = node_of_slot[sl]
        xs = np.zeros((SLOTS, IN_DIM), np.float32)
        valid = nos >= 0
        xs[valid] = x[nos[valid]]
        xT = xs.T.astype(np.float16).reshape(P, SLOTS)
        in_maps.append({
            "xT_in": xT,
            "srcg_in": srcg[c],
            "s_in": np.ascontiguousarray(s_mat[c].transpose(1, 0, 2)).reshape(P, -1),
            "st_in": np.ascontiguousarray(st_mat[c].transpose(1, 0, 2)).reshape(P, -1),
            "wa1_in": wa1, "wa2_in": wa2, "w3_in": w3, "sc3_in": sc3,
            "b1_in": b1r, "b2_in": b2r,
            "ident_in": np.eye(P, dtype=np.float16),
        })

    if T not in _CACHE:
        _CACHE[T] = _build(T)
    nc = _CACHE[T]
    res = run_bass_kernel_spmd(nc, in_maps, list(range(NCORES)))

    out = np.empty(N_NODES, np.float32)
    for c in range(NCORES):
        o = res.results[c]["out_p"]
        flat = o.T.reshape(-1)
        nos = node_of_slot[c * SLOTS:(c + 1) * SLOTS]
        valid = nos >= 0
        out[nos[valid]] = flat[valid]
    return out


# revision 5
# speedup vs baseline: 1.6084x; 1.2160x over previous
"""3-layer GAT (PyG GATConv semantics) on 8 Trainium2 NeuronCores — v3.

Strategy (dst-sharded, per-tile indirect gathers, lean compute):
- Nodes assigned to 160 degree-balanced blocks of <=128 dst slots; 20 blocks/core.
- Per layer l in {1,2}: dense phase (f16 matmuls) computes [h | alpha_src | alpha_dst]
  per block; h|alpha_src (260 f16 = 520B rows) go to the AllGather'd table,
  alpha_dst stays in SBUF per dst block.
- Edge phase per 128-edge tile: one [128,1] indirect-DMA gather of table rows by edge
  src (the only per-edge data movement; InstDMACopy dynamic path coexists with
  full-speed DVE, unlike InstDMAGatherAnt which starves it). The 0/1 selector S
  (edge->dst-slot) and its transpose St are HOST-precomputed and streamed from DRAM
  (HWDGE), so the only per-tile PE work is: adx matmul (St @ alpha_dst column, into a
  per-block [128, T*4] PSUM) and the aggregation matmul (lhsT = S, rhs = [m | ex]).
  e=leaky(as+ad), ex=exp(e) (f32->f16), m = ex*h.
- Layer 3 (heads=1, C=1): 8B f32 rows [h3 | a_src3*h3], same scheme.
- Dense phases run interleaved with the previous layer's edge blocks.

The walrus in this toolchain accepts only ONE sync wait per instruction; BassOneWait
splits Tile-generated multi-waits into single-wait EventSemaphore ops at serialization.
"""
import numpy as np
from contextlib import ExitStack
import heapq

import orjson
import concourse.bass as bass
import concourse.tile as tile
from concourse import mybir
from concourse.bass_utils import run_bass_kernel_spmd
from concourse.library_config import mlp

# problem constants (fixed by the harness's setup_inputs)
N_NODES = 20000
N_EDGES = 320000
IN_DIM = 128
HID = 64
HEADS = 4
HC = HEADS * HID          # 256
ROWW = HC + HEADS         # 260 = [h | alpha_src] (useful part)
ROWP = 384                # padded table row (768B, dma_gather 256B-alignment)
WAUG = HC + 2 * HEADS     # 264 = dense out: [h | alpha_src | alpha_dst]
NEG = 0.2
NCORES = 8
P = 128
NBLK = 20                 # dst blocks per core
SLOTS = NBLK * P          # 2560 slots per core
TOT_SLOTS = SLOTS * NCORES
CBLK = 2                  # blocks per S-chunk
NCHUNK = NBLK // CBLK

F32 = mybir.dt.float32
F16 = mybir.dt.float16
I32 = mybir.dt.int32
I16 = mybir.dt.int16

EXPF = mybir.ActivationFunctionType.Exp


def _split_multiwaits(bir: bytes) -> bytes:
    """Walrus here allows only 1 sync wait per instruction -> hoist extras onto
    same-engine EventSemaphore waits (dedup repeated ge-waits per engine; sems
    are monotonic within the block, so a repeated >= wait is a no-op)."""
    j = orjson.loads(bir)
    ctr = 0
    for fn in j["functions"]:
        for blk in fn["blocks"]:
            out_l = []
            last_wait = {}
            for ins in blk["instructions"]:
                eng = ins.get("engine")
                si = ins.get("sync_info")
                ow = (si or {}).get("on_wait") or []
                keep = 1
                if len(ow) > keep:
                    seen = last_wait.setdefault(eng, set())
                    for w in ow[:len(ow) - keep]:
                        key = (w.get("id"), w.get("wait_mode"), w.get("wait_value"))
                        if w.get("wait_mode") == "sem-ge-imm":
                            if key in seen:
                                continue
                            seen.add(key)
                        ctr += 1
                        out_l.append({
                            "engine": eng, "ins": [], "outs": [],
                            "name": f"mwsplit-{ctr}", "opcode": "EventSemaphore",
                            "sync_info": {"on_update": [], "on_wait": [w]},
                        })
                    si["on_wait"] = ow[len(ow) - keep:]
                out_l.append(ins)
            blk["instructions"] = out_l
    return orjson.dumps(j)


class BassOneWait(bass.Bass):
    def to_json_bytes(self):
        return _split_multiwaits(super().to_json_bytes())


# ---------------------------------------------------------------- host prep

def _preprocess(edge_index):
    """Assign nodes to degree-balanced blocks; build per-core edge arrays."""
    src = np.asarray(edge_index[0], dtype=np.int64)
    dst = np.asarray(edge_index[1], dtype=np.int64)
    loops = np.arange(N_NODES, dtype=np.int64)
    src = np.concatenate([src, loops])
    dst = np.concatenate([dst, loops])
    deg = np.bincount(dst, minlength=N_NODES).astype(np.int64)

    NB_TOT = NCORES * NBLK
    order = np.argsort(-deg, kind="stable")
    blk_of = np.empty(N_NODES, np.int32)
    slot_of = np.empty(N_NODES, np.int32)
    heap = [(0, 0, b) for b in range(NB_TOT)]
    heapq.heapify(heap)
    cnt = np.zeros(NB_TOT, np.int32)
    load = np.zeros(NB_TOT, np.int64)
    for n in order:
        while True:
            l, _, b = heapq.heappop(heap)
            if cnt[b] < P:
                break
        blk_of[n] = b
        slot_of[n] = cnt[b]
        cnt[b] += 1
        load[b] += deg[n]
        if cnt[b] < P:
            heapq.heappush(heap, (load[b], cnt[b], b))

    T = int(np.ceil(load.max() / P))
    gslot = blk_of.astype(np.int64) * P + slot_of
    node_of_slot = np.full(NB_TOT * P, -1, np.int64)
    node_of_slot[gslot] = np.arange(N_NODES)

    # bucket edges by dst block, sorted by src gslot for HBM locality
    eb = blk_of[dst]
    order_e = np.lexsort((gslot[src], eb))
    src_s = src[order_e]
    dst_s = dst[order_e]
    eb_s = eb[order_e]
    starts = np.searchsorted(eb_s, np.arange(NB_TOT + 1))

    NT = NBLK * T
    srcg = np.zeros((NCORES, P, NT * 8), np.int16)
    s_mat = np.zeros((NCORES, NT, P, P), np.float16)   # [tile, edge, slot]
    st_mat = np.zeros((NCORES, NT, P, P), np.float16)  # [tile, slot, edge]
    N = T * P
    TL, Pp = np.meshgrid(np.arange(T), np.arange(P), indexing="ij")
    JJ = (Pp % 16) * (N // 16) + (Pp // 16) + 8 * TL
    for b in range(NB_TOT):
        c, lb = divmod(b, NBLK)
        e0, e1 = starts[b], starts[b + 1]
        k = e1 - e0
        col = np.zeros(T * P, np.int64)
        col[:k] = gslot[src_s[e0:e1]]
        rows = col.reshape(T, P)            # [t, p] -> src row
        flat = np.zeros(N, np.int64)
        flat[JJ] = rows[TL, Pp]
        srcg[c, :, lb * (N // 16):(lb + 1) * (N // 16)] = np.tile(
            flat.reshape(16, N // 16), (8, 1)).astype(np.int16)
        sl = np.full(T * P, -1, np.int64)
        sl[:k] = slot_of[dst_s[e0:e1]]
        sl = sl.reshape(T, P)
        tt, ee = np.nonzero(sl >= 0)
        s_mat[c, lb * T + tt, ee, sl[tt, ee]] = 1.0
        st_mat[c, lb * T + tt, sl[tt, ee], ee] = 1.0

    return T, gslot, node_of_slot, srcg, s_mat, st_mat


def _aug_weights(W, a_src, a_dst, heads, hid):
    """[W | ws | wd], f16: ws[:,h] = W[:,h*hid:(h+1)*hid] @ a_src[h]."""
    cin = W.shape[0]
    ws = np.zeros((cin, heads), np.float32)
    wd = np.zeros((cin, heads), np.float32)
    for h in range(heads):
        blk = W[:, h * hid:(h + 1) * hid]
        ws[:, h] = blk @ a_src[h]
        wd[:, h] = blk @ a_dst[h]
    return np.concatenate([W, ws, wd], axis=1).astype(np.float16)


# ---------------------------------------------------------------- device kernel

def _build(T):
    NT = NBLK * T
    CT = CBLK * T
    nc = BassOneWait()
    dp = nc.declare_dram_parameter
    xT_in = dp("xT_in", [P, NBLK * P], F16, isOutput=False)
    srcg_in = dp("srcg_in", [P, NT * 8], I16, isOutput=False)
    s_in = dp("s_in", [P, NT * P], F16, isOutput=False)
    st_in = dp("st_in", [P, NT * P], F16, isOutput=False)
    wa1_in = dp("wa1_in", [IN_DIM, WAUG], F16, isOutput=False)
    wa2_in = dp("wa2_in", [HC, WAUG], F16, isOutput=False)
    w3_in = dp("w3_in", [1, HC], F16, isOutput=False)
    sc3_in = dp("sc3_in", [1, 4], F32, isOutput=False)
    b1_in = dp("b1_in", [1, HC], F32, isOutput=False)
    b2_in = dp("b2_in", [1, HC], F32, isOutput=False)
    ident_in = dp("ident_in", [P, P], F16, isOutput=False)
    out_p = dp("out_p", [P, NBLK], F32, isOutput=True)

    tab_sh = [nc.dram_tensor(f"tab_sh{l}", [SLOTS, ROWP], F16) for l in (1, 2)]
    tab_full = [nc.dram_tensor(f"tab_full{l}", [TOT_SLOTS, ROWP], F16) for l in (1, 2)]
    tab3_sh = nc.dram_tensor("tab3_sh", [SLOTS, 128], F16)
    tab3_full = nc.dram_tensor("tab3_full", [TOT_SLOTS, 128], F16)

    groups = [list(range(NCORES))]

    with tile.TileContext(nc) as tc, ExitStack() as ctx:
        consts = ctx.enter_context(tc.tile_pool(name="consts", bufs=1))
        meta = ctx.enter_context(tc.tile_pool(name="meta", bufs=1))
        spool = ctx.enter_context(tc.tile_pool(name="spool", bufs=3))
        gpool = ctx.enter_context(tc.tile_pool(name="gpool", bufs=4))
        mpool = ctx.enter_context(tc.tile_pool(name="mpool", bufs=2))
        small = ctx.enter_context(tc.tile_pool(name="small", bufs=2))
        sttp = ctx.enter_context(tc.tile_pool(name="sttp", bufs=3))
        psd = ctx.enter_context(tc.tile_pool(name="psd", bufs=2, space="PSUM"))
        pse = ctx.enter_context(tc.tile_pool(name="pse", bufs=2, space="PSUM"))
        pst = ctx.enter_context(tc.tile_pool(name="pst", bufs=2, space="PSUM"))
        psa = ctx.enter_context(tc.tile_pool(name="psa", bufs=2, space="PSUM"))

        nc.gpsimd.load_library(mlp)
        nidx_reg = nc.gpsimd.to_reg(T * P)

        # ---- constants / metadata
        ident16 = consts.tile([P, P], F16)
        nc.sync.dma_start(out=ident16, in_=ident_in[:])
        wa1 = consts.tile([P, WAUG], F16)
        nc.sync.dma_start(out=wa1, in_=wa1_in[:])
        wa2 = consts.tile([P, 2, WAUG], F16)
        nc.sync.dma_start(out=wa2, in_=wa2_in.rearrange("(j p) a -> p j a", p=P))

        def rep_load(name, src, n, dt):
            t = consts.tile([P, n], dt, tag=name)
            bc = bass.AP(tensor=src.tensor, offset=0, ap=[[0, P], [1, n]])
            nc.sync.dma_start(out=t, in_=bc)
            return t
        w3r = rep_load("w3r", w3_in[:], HC, F16)
        sc3 = rep_load("sc3", sc3_in[:], 4, F32)
        b1r = rep_load("b1r", b1_in[:], HC, F32)
        b2r = rep_load("b2r", b2_in[:], HC, F32)

        srcg = meta.tile([P, NT * 8], I16)
        nc.sync.dma_start(out=srcg, in_=srcg_in[:])
        hT = meta.tile([P, 2 * NBLK, P], F16)
        nc.sync.dma_start(out=hT[:, 0:NBLK, :],
                          in_=xT_in.rearrange("p (b n) -> p b n", n=P))
        outsb = meta.tile([P, NBLK], F32)
        adl = meta.tile([P, NBLK, HEADS], F16, tag="adl")    # layer 1/2 alpha_dst
        adl3 = meta.tile([P, NBLK, 1], F16, tag="adl3")

        def bcast_row(t, shape):
            ap = [list(t.ap[0])]
            for s in shape[1:-1]:
                ap.append([0, s])
            ap.append([t.ap[-1][0], shape[-1]])
            return bass.AP(tensor=t.tensor, offset=t.offset, ap=ap)

        def dense_block(lidx, b):
            ps = psd.tile([P, WAUG], F32, tag="dense")
            if lidx == 0:
                nc.tensor.matmul(ps, hT[:, b, :], wa1, start=True, stop=True)
            else:
                nc.tensor.matmul(ps, hT[:, 2 * b, :], wa2[:, 0, :],
                                 start=True, stop=False)
                nc.tensor.matmul(ps, hT[:, 2 * b + 1, :], wa2[:, 1, :],
                                 start=False, stop=True)
            tabt = small.tile([P, ROWW], F16, tag="tabt")
            nc.vector.tensor_copy(out=tabt, in_=ps[:, 0:ROWW])
            nc.sync.dma_start(
                out=tab_sh[lidx].rearrange("(b p) a -> p b a", p=P)[:, b, 0:ROWW],
                in_=tabt)
            nc.vector.tensor_copy(out=adl[:, b, :], in_=ps[:, ROWW:WAUG])

        s_view = s_in.rearrange("p (n q) -> p n q", q=P)    # [P, NT, P]: tile n -> [e, slot]
        st_view = st_in.rearrange("p (n q) -> p n q", q=P)

        def load_s(b):
            S = spool.tile([P, T, P], F16, tag="S")
            nc.sync.dma_start(out=S, in_=s_view[:, b * T:(b + 1) * T, :])
            St = sttp.tile([P, T, P], F16, tag="St")
            nc.sync.dma_start(out=St, in_=st_view[:, b * T:(b + 1) * T, :])
            return S, St

        def edge12(lidx, brow, after_block):
            for b in range(NBLK):
                    S, St = load_s(b)
                    hg = gpool.tile([P, T, ROWP], F16, tag="hg")
                    iw = T * 8
                    nc.gpsimd.dma_gather(hg, tab_full[lidx][:],
                                         srcg[:, b * iw:(b + 1) * iw],
                                         T * P, nidx_reg, ROWP,
                                         single_packet=False)
                    # per-edge alpha_dst: St_t @ block's alpha_dst column
                    adx = psa.tile([P, T, HEADS], F32, tag="adx")
                    for t in range(T):
                        nc.tensor.matmul(adx[:, t, :], St[:, t, :], adl[:, b, :],
                                         start=True, stop=True)
                    asum = small.tile([P, T, HEADS], F32, tag="asum")
                    nc.vector.tensor_tensor(out=asum, in0=adx,
                                            in1=hg[:, :, HC:ROWW],
                                            op=mybir.AluOpType.add)
                    lk = small.tile([P, T, HEADS], F32, tag="lk")
                    nc.vector.tensor_scalar_mul(lk, asum, NEG)
                    nc.vector.tensor_tensor(out=lk, in0=lk, in1=asum,
                                            op=mybir.AluOpType.max)
                    exf = small.tile([P, T, HEADS], F16, tag="exf")
                    nc.scalar.activation(out=exf, in_=lk, func=EXPF)
                    m = mpool.tile([P, T, ROWW], F16, tag="m")
                    ex_b = bass.AP(tensor=exf.tensor, offset=exf.offset,
                                   ap=[exf.ap[0], exf.ap[1], exf.ap[2], [0, HID]])
                    nc.vector.tensor_tensor(
                        out=m[:, :, 0:HC].rearrange("p t (h k) -> p t h k", h=HEADS),
                        in0=hg[:, :, 0:HC].rearrange("p t (h k) -> p t h k", h=HEADS),
                        in1=ex_b, op=mybir.AluOpType.mult)
                    nc.vector.tensor_copy(out=m[:, :, HC:ROWW], in_=exf)

                    ps = pse.tile([P, ROWW], F32, tag="agg")
                    for t in range(T):
                        nc.tensor.matmul(ps, S[:, t, :], m[:, t, :],
                                         start=(t == 0), stop=(t == T - 1))

                    den = small.tile([P, HEADS], F32, tag="den")
                    nc.vector.tensor_scalar_max(den, ps[:, HC:ROWW], 1e-30)
                    rec = small.tile([P, HEADS], F32, tag="rec")
                    nc.vector.reciprocal(out=rec, in_=den)
                    rec_b = bass.AP(tensor=rec.tensor, offset=rec.offset,
                                    ap=[rec.ap[0], rec.ap[1], [0, HID]])
                    hn = small.tile([P, HC], F32, tag="hn")
                    nc.vector.tensor_tensor(
                        out=hn.rearrange("p (h k) -> p h k", h=HEADS),
                        in0=ps[:, 0:HC].rearrange("p (h k) -> p h k", h=HEADS),
                        in1=rec_b, op=mybir.AluOpType.mult)
                    nc.vector.tensor_tensor(out=hn, in0=hn, in1=brow,
                                            op=mybir.AluOpType.add)
                    emin = small.tile([P, HC], F32, tag="emin")
                    nc.vector.tensor_scalar_min(emin, hn, 0.0)
                    eex = small.tile([P, HC], F32, tag="eex")
                    nc.scalar.activation(out=eex, in_=emin, func=EXPF)
                    nc.vector.tensor_scalar_max(hn, hn, 0.0)
                    nc.vector.tensor_tensor(out=hn, in0=hn, in1=eex,
                                            op=mybir.AluOpType.add)
                    hn16 = small.tile([P, HC], F16, tag="hn16")
                    nc.vector.tensor_scalar_add(hn16, hn, -1.0)
                    after_block(b, hn16)

        # ---------------- layer 1 dense + AllGather
        for b in range(NBLK):
            dense_block(0, b)
        nc.gpsimd.collective_compute(
            "AllGather", mybir.AluOpType.bypass, replica_groups=groups,
            ins=[tab_sh[0][:]], outs=[tab_full[0][:]])

        # ---------------- layer 1 edge (+ layer 2 dense interleaved)
        def after1(b, hn16):
            tp = pst.tile([P, P], F16, tag="tr")
            nc.tensor.transpose(out=tp, in_=hn16[:, 0:P], identity=ident16)
            nc.vector.tensor_copy(out=hT[:, 2 * b, :], in_=tp)
            tp2 = pst.tile([P, P], F16, tag="tr")
            nc.tensor.transpose(out=tp2, in_=hn16[:, P:HC], identity=ident16)
            nc.vector.tensor_copy(out=hT[:, 2 * b + 1, :], in_=tp2)
            dense_block(1, b)
        edge12(0, b1r, after1)
        nc.gpsimd.collective_compute(
            "AllGather", mybir.AluOpType.bypass, replica_groups=groups,
            ins=[tab_sh[1][:]], outs=[tab_full[1][:]])

        # ---------------- layer 2 edge (+ layer 3 dense inline)
        def after2(b, hn16):
            t3 = small.tile([P, HC], F16, tag="t3")
            nc.vector.tensor_tensor(out=t3, in0=hn16, in1=w3r,
                                    op=mybir.AluOpType.mult)
            h3 = small.tile([P, 1], F32, tag="h3")
            nc.vector.tensor_reduce(out=h3, in_=t3, axis=mybir.AxisListType.X,
                                    op=mybir.AluOpType.add)
            row3 = small.tile([P, 2], F16, tag="row3")
            nc.vector.tensor_copy(out=row3[:, 0:1], in_=h3)
            nc.vector.tensor_tensor(out=row3[:, 1:2], in0=h3, in1=sc3[:, 0:1],
                                    op=mybir.AluOpType.mult)
            nc.sync.dma_start(
                out=tab3_sh.rearrange("(b p) a -> p b a", p=P)[:, b, 0:2],
                in_=row3)
            nc.vector.tensor_tensor(out=adl3[:, b, :], in0=h3, in1=sc3[:, 1:2],
                                    op=mybir.AluOpType.mult)
        edge12(1, b2r, after2)
        nc.gpsimd.collective_compute(
            "AllGather", mybir.AluOpType.bypass, replica_groups=groups,
            ins=[tab3_sh[:]], outs=[tab3_full[:]])

        # ---------------- layer 3 edge
        for b in range(NBLK):
                S, St = load_s(b)
                g3 = gpool.tile([P, T, 128], F16, tag="g3")
                iw = T * 8
                nc.gpsimd.dma_gather(g3, tab3_full[:],
                                     srcg[:, b * iw:(b + 1) * iw],
                                     T * P, nidx_reg, 128,
                                     single_packet=False)
                d3 = psa.tile([P, T, HEADS], F32, tag="adx")
                for t in range(T):
                    nc.tensor.matmul(d3[:, t, 0:1], St[:, t, :], adl3[:, b, :],
                                     start=True, stop=True)
                e3 = small.tile([P, T, 1], F32, tag="e3")
                nc.vector.tensor_tensor(out=e3, in0=g3[:, :, 1:2],
                                        in1=d3[:, :, 0:1],
                                        op=mybir.AluOpType.add)
                lk3 = small.tile([P, T, 1], F32, tag="lk3")
                nc.vector.tensor_scalar_mul(lk3, e3, NEG)
                nc.vector.tensor_tensor(out=lk3, in0=lk3, in1=e3,
                                        op=mybir.AluOpType.max)
                ex3 = small.tile([P, T, 1], F32, tag="ex3")
                nc.scalar.activation(out=ex3, in_=lk3, func=EXPF)
                m3 = small.tile([P, T, 2], F16, tag="m3")
                nc.vector.tensor_tensor(out=m3[:, :, 0:1], in0=ex3,
                                        in1=g3[:, :, 0:1],
                                        op=mybir.AluOpType.mult)
                nc.vector.tensor_copy(out=m3[:, :, 1:2], in_=ex3)
                ps3f = pse.tile([P, ROWW], F32, tag="agg")
                ps3 = ps3f[:, 0:2]
                for t in range(T):
                    nc.tensor.matmul(ps3, S[:, t, :], m3[:, t, :],
                                     start=(t == 0), stop=(t == T - 1))
                den3 = small.tile([P, 1], F32, tag="den3")
                nc.vector.tensor_scalar_max(den3, ps3[:, 1:2], 1e-30)
                rec3 = small.tile([P, 1], F32, tag="rec3")
                nc.vector.reciprocal(out=rec3, in_=den3)
                nc.vector.tensor_tensor(out=outsb[:, b:b + 1], in0=ps3[:, 0:1],
                                        in1=rec3, op=mybir.AluOpType.mult)
        nc.vector.tensor_tensor(out=outsb, in0=outsb,
                                in1=bass.AP(tensor=sc3.tensor,
                                            offset=sc3[:, 2:3].offset,
                                            ap=[list(sc3.ap[0]), [0, NBLK]]),
                                op=mybir.AluOpType.add)
        nc.sync.dma_start(out=out_p[:], in_=outsb)

    mybir.codegen_inst_isa_subclasses(nc)
    return nc


_CACHE = {}


def kernel(x, edge_index, W1, a_src1, a_dst1, b1, W2, a_src2, a_dst2, b2,
           W3, a_src3, a_dst3, b3):
    T, gslot, node_of_slot, srcg, s_mat, st_mat = _preprocess(np.asarray(edge_index))

    wa1 = _aug_weights(np.asarray(W1, np.float32), np.asarray(a_src1, np.float32),
                       np.asarray(a_dst1, np.float32), HEADS, HID)
    wa2 = _aug_weights(np.asarray(W2, np.float32), np.asarray(a_src2, np.float32),
                       np.asarray(a_dst2, np.float32), HEADS, HID)
    w3 = np.asarray(W3, np.float32).reshape(1, HC).astype(np.float16)
    sc3 = np.array([[float(np.asarray(a_src3).reshape(-1)[0]),
                     float(np.asarray(a_dst3).reshape(-1)[0]),
                     float(np.asarray(b3).reshape(-1)[0]), 0.0]], np.float32)
    b1r = np.asarray(b1, np.float32).reshape(1, HC)
    b2r = np.asarray(b2, np.float32).reshape(1, HC)

    x = np.asarray(x, np.float32)
    in_maps = []
    for c in range(NCORES):
        sl = slice(c * SLOTS, (c + 1) * SLOTS)
        nos = node_of_slot[sl]
        xs = np.zeros((SLOTS, IN_DIM), np.float32)
        valid = nos >= 0
        xs[valid] = x[nos[valid]]
        xT = xs.T.astype(np.float16).reshape(P, SLOTS)
        in_maps.append({
            "xT_in": xT,
            "srcg_in": srcg[c],
            "s_in": np.ascontiguousarray(s_mat[c].transpose(1, 0, 2)).reshape(P, -1),
            "st_in": np.ascontiguousarray(st_mat[c].transpose(1, 0, 2)).reshape(P, -1),
            "wa1_in": wa1, "wa2_in": wa2, "w3_in": w3, "sc3_in": sc3,
            "b1_in": b1r, "b2_in": b2r,
            "ident_in": np.eye(P, dtype=np.float16),
        })

    if T not in _CACHE:
        _CACHE[T] = _build(T)
    nc = _CACHE[T]
    res = run_bass_kernel_spmd(nc, in_maps, list(range(NCORES)))

    out = np.empty(N_NODES, np.float32)
    for c in range(NCORES):
        o = res.results[c]["out_p"]
        flat = o.T.reshape(-1)
        nos = node_of_slot[c * SLOTS:(c + 1) * SLOTS]
        valid = nos >= 0
        out[nos[valid]] = flat[valid]
    return out


# revision 7
# speedup vs baseline: 1.6895x; 1.0504x over previous
"""3-layer GAT (PyG GATConv semantics) on 8 Trainium2 NeuronCores — v3.

Strategy (dst-sharded, per-tile indirect gathers, lean compute):
- Nodes assigned to 160 degree-balanced blocks of <=128 dst slots; 20 blocks/core.
- Per layer l in {1,2}: dense phase (f16 matmuls) computes [h | alpha_src | alpha_dst]
  per block; h|alpha_src (260 f16 = 520B rows) go to the AllGather'd table,
  alpha_dst stays in SBUF per dst block.
- Edge phase per 128-edge tile: one [128,1] indirect-DMA gather of table rows by edge
  src (the only per-edge data movement; InstDMACopy dynamic path coexists with
  full-speed DVE, unlike InstDMAGatherAnt which starves it). The 0/1 selector S
  (edge->dst-slot) and its transpose St are HOST-precomputed and streamed from DRAM
  (HWDGE), so the only per-tile PE work is: adx matmul (St @ alpha_dst column, into a
  per-block [128, T*4] PSUM) and the aggregation matmul (lhsT = S, rhs = [m | ex]).
  e=leaky(as+ad), ex=exp(e) (f32->f16), m = ex*h.
- Layer 3 (heads=1, C=1): 8B f32 rows [h3 | a_src3*h3], same scheme.
- Dense phases run interleaved with the previous layer's edge blocks.

The walrus in this toolchain accepts only ONE sync wait per instruction; BassOneWait
splits Tile-generated multi-waits into single-wait EventSemaphore ops at serialization.
"""
import numpy as np
from contextlib import ExitStack
import heapq

import orjson
import concourse.bass as bass
import concourse.tile as tile
from concourse import mybir
from concourse.bass_utils import run_bass_kernel_spmd
from concourse.library_config import mlp

# problem constants (fixed by the harness's setup_inputs)
N_NODES = 20000
N_EDGES = 320000
IN_DIM = 128
HID = 64
HEADS = 4
HC = HEADS * HID          # 256
ROWW = HC + HEADS         # 260 = [h | alpha_src] (useful part)
ROWP = 384                # padded table row (768B, dma_gather 256B-alignment)
WAUG = HC + 2 * HEADS     # 264 = dense out: [h | alpha_src | alpha_dst]
NEG = 0.2
NCORES = 8
P = 128
NBLK = 20                 # dst blocks per core
SLOTS = NBLK * P          # 2560 slots per core
TOT_SLOTS = SLOTS * NCORES
CBLK = 2                  # blocks per S-chunk
NCHUNK = NBLK // CBLK

F32 = mybir.dt.float32
F16 = mybir.dt.float16
I32 = mybir.dt.int32
I16 = mybir.dt.int16

EXPF = mybir.ActivationFunctionType.Exp
LRELU = mybir.ActivationFunctionType.Lrelu
RELU = mybir.ActivationFunctionType.Relu
COPYF = mybir.ActivationFunctionType.Copy


def _split_multiwaits(bir: bytes) -> bytes:
    """Walrus here allows only 1 sync wait per instruction -> hoist extras onto
    same-engine EventSemaphore waits (dedup repeated ge-waits per engine; sems
    are monotonic within the block, so a repeated >= wait is a no-op)."""
    j = orjson.loads(bir)
    ctr = 0
    for fn in j["functions"]:
        for blk in fn["blocks"]:
            out_l = []
            last_wait = {}
            for ins in blk["instructions"]:
                eng = ins.get("engine")
                si = ins.get("sync_info")
                ow = (si or {}).get("on_wait") or []
                keep = 1
                if len(ow) > keep:
                    seen = last_wait.setdefault(eng, set())
                    for w in ow[:len(ow) - keep]:
                        key = (w.get("id"), w.get("wait_mode"), w.get("wait_value"))
                        if w.get("wait_mode") == "sem-ge-imm":
                            if key in seen:
                                continue
                            seen.add(key)
                        ctr += 1
                        out_l.append({
                            "engine": eng, "ins": [], "outs": [],
                            "name": f"mwsplit-{ctr}", "opcode": "EventSemaphore",
                            "sync_info": {"on_update": [], "on_wait": [w]},
                        })
                    si["on_wait"] = ow[len(ow) - keep:]
                out_l.append(ins)
            blk["instructions"] = out_l
    return orjson.dumps(j)


class BassOneWait(bass.Bass):
    def to_json_bytes(self):
        return _split_multiwaits(super().to_json_bytes())


# ---------------------------------------------------------------- host prep

def _preprocess(edge_index):
    """Assign nodes to degree-balanced blocks; build per-core edge arrays."""
    src = np.asarray(edge_index[0], dtype=np.int64)
    dst = np.asarray(edge_index[1], dtype=np.int64)
    loops = np.arange(N_NODES, dtype=np.int64)
    src = np.concatenate([src, loops])
    dst = np.concatenate([dst, loops])
    deg = np.bincount(dst, minlength=N_NODES).astype(np.int64)

    NB_TOT = NCORES * NBLK
    order = np.argsort(-deg, kind="stable")
    blk_of = np.empty(N_NODES, np.int32)
    slot_of = np.empty(N_NODES, np.int32)
    heap = [(0, 0, b) for b in range(NB_TOT)]
    heapq.heapify(heap)
    cnt = np.zeros(NB_TOT, np.int32)
    load = np.zeros(NB_TOT, np.int64)
    for n in order:
        while True:
            l, _, b = heapq.heappop(heap)
            if cnt[b] < P:
                break
        blk_of[n] = b
        slot_of[n] = cnt[b]
        cnt[b] += 1
        load[b] += deg[n]
        if cnt[b] < P:
            heapq.heappush(heap, (load[b], cnt[b], b))

    T = int(np.ceil(load.max() / P))
    gslot = blk_of.astype(np.int64) * P + slot_of
    node_of_slot = np.full(NB_TOT * P, -1, np.int64)
    node_of_slot[gslot] = np.arange(N_NODES)
    # table-row id under the split-AllGather layout:
    # rows [0:HTOT) = all cores' blocks 0..9, [HTOT:) = blocks 10..19
    core_of = blk_of // NBLK
    lb_of = blk_of % NBLK
    half = (lb_of >= NBLK // 2).astype(np.int64)
    growt = (half * (NB_TOT * P // 2) + core_of * (NBLK // 2) * P
             + (lb_of - half * (NBLK // 2)) * P + slot_of)

    # bucket edges by dst block, sorted by src gslot for HBM locality
    eb = blk_of[dst]
    order_e = np.lexsort((gslot[src], eb))
    src_s = src[order_e]
    dst_s = dst[order_e]
    eb_s = eb[order_e]
    starts = np.searchsorted(eb_s, np.arange(NB_TOT + 1))

    NT = NBLK * T
    srcg = np.zeros((NCORES, P, NT * 8), np.int16)
    s_mat = np.zeros((NCORES, NT, P, P), np.float16)   # [tile, edge, slot]
    st_mat = np.zeros((NCORES, NT, P, P), np.float16)  # [tile, slot, edge]
    N = T * P
    TL, Pp = np.meshgrid(np.arange(T), np.arange(P), indexing="ij")
    JJ = (Pp % 16) * (N // 16) + (Pp // 16) + 8 * TL
    for b in range(NB_TOT):
        c, lb = divmod(b, NBLK)
        e0, e1 = starts[b], starts[b + 1]
        k = e1 - e0
        col = np.zeros(T * P, np.int64)
        col[:k] = growt[src_s[e0:e1]]
        rows = col.reshape(T, P)            # [t, p] -> src row
        flat = np.zeros(N, np.int64)
        flat[JJ] = rows[TL, Pp]
        srcg[c, :, lb * (N // 16):(lb + 1) * (N // 16)] = np.tile(
            flat.reshape(16, N // 16), (8, 1)).astype(np.int16)
        sl = np.full(T * P, -1, np.int64)
        sl[:k] = slot_of[dst_s[e0:e1]]
        sl = sl.reshape(T, P)
        tt, ee = np.nonzero(sl >= 0)
        s_mat[c, lb * T + tt, ee, sl[tt, ee]] = 1.0
        st_mat[c, lb * T + tt, sl[tt, ee], ee] = 1.0

    return T, gslot, node_of_slot, srcg, s_mat, st_mat


def _aug_weights(W, a_src, a_dst, heads, hid):
    """[W | ws | wd], f16: ws[:,h] = W[:,h*hid:(h+1)*hid] @ a_src[h]."""
    cin = W.shape[0]
    ws = np.zeros((cin, heads), np.float32)
    wd = np.zeros((cin, heads), np.float32)
    for h in range(heads):
        blk = W[:, h * hid:(h + 1) * hid]
        ws[:, h] = blk @ a_src[h]
        wd[:, h] = blk @ a_dst[h]
    return np.concatenate([W, ws, wd], axis=1).astype(np.float16)


# ---------------------------------------------------------------- device kernel

def _build(T):
    NT = NBLK * T
    CT = CBLK * T
    nc = BassOneWait()
    dp = nc.declare_dram_parameter
    xT_in = dp("xT_in", [P, NBLK * P], F16, isOutput=False)
    srcg_in = dp("srcg_in", [P, NT * 8], I16, isOutput=False)
    s_in = dp("s_in", [P, NT * P], F16, isOutput=False)
    st_in = dp("st_in", [P, NT * P], F16, isOutput=False)
    wa1_in = dp("wa1_in", [IN_DIM, WAUG], F16, isOutput=False)
    wa2_in = dp("wa2_in", [HC, WAUG], F16, isOutput=False)
    w3_in = dp("w3_in", [1, HC], F16, isOutput=False)
    sc3_in = dp("sc3_in", [1, 4], F32, isOutput=False)
    b1_in = dp("b1_in", [1, HC], F32, isOutput=False)
    b2_in = dp("b2_in", [1, HC], F32, isOutput=False)
    ident_in = dp("ident_in", [P, P], F16, isOutput=False)
    out_p = dp("out_p", [P, NBLK], F32, isOutput=True)

    tab_sh = [nc.dram_tensor(f"tab_sh{l}", [SLOTS, ROWP], F16) for l in (1, 2)]
    tab_full = [nc.dram_tensor(f"tab_full{l}", [TOT_SLOTS, ROWP], F16) for l in (1, 2)]
    tab3_sh = nc.dram_tensor("tab3_sh", [SLOTS, 128], F16)
    tab3_full = nc.dram_tensor("tab3_full", [TOT_SLOTS, 128], F16)

    groups = [list(range(NCORES))]

    with tile.TileContext(nc) as tc, ExitStack() as ctx:
        consts = ctx.enter_context(tc.tile_pool(name="consts", bufs=1))
        meta = ctx.enter_context(tc.tile_pool(name="meta", bufs=1))
        spool = ctx.enter_context(tc.tile_pool(name="spool", bufs=3))
        gpool = ctx.enter_context(tc.tile_pool(name="gpool", bufs=4))
        mpool = ctx.enter_context(tc.tile_pool(name="mpool", bufs=2))
        small = ctx.enter_context(tc.tile_pool(name="small", bufs=2))
        sttp = ctx.enter_context(tc.tile_pool(name="sttp", bufs=3))
        psd = ctx.enter_context(tc.tile_pool(name="psd", bufs=2, space="PSUM"))
        pse = ctx.enter_context(tc.tile_pool(name="pse", bufs=2, space="PSUM"))
        pst = ctx.enter_context(tc.tile_pool(name="pst", bufs=2, space="PSUM"))
        psa = ctx.enter_context(tc.tile_pool(name="psa", bufs=2, space="PSUM"))

        nc.gpsimd.load_library(mlp)
        nidx_reg = nc.gpsimd.to_reg(T * P)

        # ---- constants / metadata
        ident16 = consts.tile([P, P], F16)
        nc.sync.dma_start(out=ident16, in_=ident_in[:])
        wa1 = consts.tile([P, WAUG], F16)
        nc.sync.dma_start(out=wa1, in_=wa1_in[:])
        wa2 = consts.tile([P, 2, WAUG], F16)
        nc.sync.dma_start(out=wa2, in_=wa2_in.rearrange("(j p) a -> p j a", p=P))

        def rep_load(name, src, n, dt):
            t = consts.tile([P, n], dt, tag=name)
            bc = bass.AP(tensor=src.tensor, offset=0, ap=[[0, P], [1, n]])
            nc.sync.dma_start(out=t, in_=bc)
            return t
        w3r = rep_load("w3r", w3_in[:], HC, F16)
        sc3 = rep_load("sc3", sc3_in[:], 4, F32)
        b1r = rep_load("b1r", b1_in[:], HC, F32)
        b2r = rep_load("b2r", b2_in[:], HC, F32)

        srcg = meta.tile([P, NT * 8], I16)
        nc.sync.dma_start(out=srcg, in_=srcg_in[:])
        hT = meta.tile([P, 2 * NBLK, P], F16)
        nc.sync.dma_start(out=hT[:, 0:NBLK, :],
                          in_=xT_in.rearrange("p (b n) -> p b n", n=P))
        outsb = meta.tile([P, NBLK], F32)
        adl = meta.tile([P, NBLK, HEADS], F16, tag="adl")    # layer 1/2 alpha_dst
        adl3 = meta.tile([P, NBLK, 1], F16, tag="adl3")

        def bcast_row(t, shape):
            ap = [list(t.ap[0])]
            for s in shape[1:-1]:
                ap.append([0, s])
            ap.append([t.ap[-1][0], shape[-1]])
            return bass.AP(tensor=t.tensor, offset=t.offset, ap=ap)

        def dense_block(lidx, b):
            ps = psd.tile([P, WAUG], F32, tag="dense")
            if lidx == 0:
                nc.tensor.matmul(ps, hT[:, b, :], wa1, start=True, stop=True)
            else:
                nc.tensor.matmul(ps, hT[:, 2 * b, :], wa2[:, 0, :],
                                 start=True, stop=False)
                nc.tensor.matmul(ps, hT[:, 2 * b + 1, :], wa2[:, 1, :],
                                 start=False, stop=True)
            tabt = small.tile([P, ROWW], F16, tag="tabt")
            nc.scalar.activation(out=tabt, in_=ps[:, 0:ROWW], func=COPYF)
            nc.sync.dma_start(
                out=tab_sh[lidx].rearrange("(b p) a -> p b a", p=P)[:, b, 0:ROWW],
                in_=tabt)
            nc.scalar.activation(out=adl[:, b, :], in_=ps[:, ROWW:WAUG], func=COPYF)

        s_view = s_in.rearrange("p (n q) -> p n q", q=P)    # [P, NT, P]: tile n -> [e, slot]
        st_view = st_in.rearrange("p (n q) -> p n q", q=P)

        def load_s(b):
            S = spool.tile([P, T, P], F16, tag="S")
            nc.sync.dma_start(out=S, in_=s_view[:, b * T:(b + 1) * T, :])
            St = sttp.tile([P, T, P], F16, tag="St")
            nc.sync.dma_start(out=St, in_=st_view[:, b * T:(b + 1) * T, :])
            return S, St

        def edge12(lidx, brow, after_block, mid_hook=None):
            for b in range(NBLK):
                    S, St = load_s(b)
                    hg = gpool.tile([P, T, ROWP], F16, tag="hg")
                    iw = T * 8
                    nc.gpsimd.dma_gather(hg, tab_full[lidx][:],
                                         srcg[:, b * iw:(b + 1) * iw],
                                         T * P, nidx_reg, ROWP,
                                         single_packet=False)
                    # per-edge alpha_dst: St_t @ block's alpha_dst column
                    adx = psa.tile([P, T, HEADS], F32, tag="adx")
                    for t in range(T):
                        nc.tensor.matmul(adx[:, t, :], St[:, t, :], adl[:, b, :],
                                         start=True, stop=True)
                    asum = small.tile([P, T, HEADS], F32, tag="asum")
                    nc.vector.tensor_tensor(out=asum, in0=adx,
                                            in1=hg[:, :, HC:ROWW],
                                            op=mybir.AluOpType.add)
                    lk = small.tile([P, T, HEADS], F32, tag="lk")
                    nc.vector.tensor_scalar_mul(lk, asum, NEG)
                    nc.vector.tensor_tensor(out=lk, in0=lk, in1=asum,
                                            op=mybir.AluOpType.max)
                    exf = small.tile([P, T, HEADS], F16, tag="exf")
                    nc.scalar.activation(out=exf, in_=lk, func=EXPF)
                    m = mpool.tile([P, T, ROWW], F16, tag="m")
                    ex_b = bass.AP(tensor=exf.tensor, offset=exf.offset,
                                   ap=[exf.ap[0], exf.ap[1], exf.ap[2], [0, HID]])
                    nc.vector.tensor_tensor(
                        out=m[:, :, 0:HC].rearrange("p t (h k) -> p t h k", h=HEADS),
                        in0=hg[:, :, 0:HC].rearrange("p t (h k) -> p t h k", h=HEADS),
                        in1=ex_b, op=mybir.AluOpType.mult)
                    nc.scalar.activation(out=m[:, :, HC:ROWW], in_=exf, func=COPYF)

                    ps = pse.tile([P, ROWW], F32, tag="agg")
                    for t in range(T):
                        nc.tensor.matmul(ps, S[:, t, :], m[:, t, :],
                                         start=(t == 0), stop=(t == T - 1))

                    den = small.tile([P, HEADS], F32, tag="den")
                    nc.scalar.activation(out=den, in_=ps[:, HC:ROWW], func=COPYF,
                                         bias=1e-30)
                    rec = small.tile([P, HEADS], F32, tag="rec")
                    nc.vector.reciprocal(out=rec, in_=den)
                    rec_b = bass.AP(tensor=rec.tensor, offset=rec.offset,
                                    ap=[rec.ap[0], rec.ap[1], [0, HID]])
                    hn = small.tile([P, HC], F32, tag="hn")
                    nc.vector.tensor_tensor(
                        out=hn.rearrange("p (h k) -> p h k", h=HEADS),
                        in0=ps[:, 0:HC].rearrange("p (h k) -> p h k", h=HEADS),
                        in1=rec_b, op=mybir.AluOpType.mult)
                    nc.vector.tensor_tensor(out=hn, in0=hn, in1=brow,
                                            op=mybir.AluOpType.add)
                    emin = small.tile([P, HC], F32, tag="emin")
                    nc.scalar.activation(out=emin, in_=hn, func=RELU, scale=-1.0)
                    eex = small.tile([P, HC], F32, tag="eex")
                    nc.scalar.activation(out=eex, in_=emin, func=EXPF, scale=-1.0)
                    hnp = small.tile([P, HC], F32, tag="hnp")
                    nc.scalar.activation(out=hnp, in_=hn, func=RELU)
                    nc.vector.tensor_tensor(out=hn, in0=hnp, in1=eex,
                                            op=mybir.AluOpType.add)
                    hn16 = small.tile([P, HC], F16, tag="hn16")
                    nc.vector.tensor_scalar_add(hn16, hn, -1.0)
                    after_block(b, hn16)
                    if mid_hook is not None and b == NBLK // 2 - 1:
                        mid_hook()

        # ---------------- layer 1 dense + AllGather (split into halves:
        # tab_full rows [0:HTOT) = all cores' blocks 0..9, [HTOT:) = 10..19)
        HSH = SLOTS // 2
        HTOT = TOT_SLOTS // 2
        def ag(tsh, tfull, half):
            nc.gpsimd.collective_compute(
                "AllGather", mybir.AluOpType.bypass, replica_groups=groups,
                ins=[tsh[half * HSH:(half + 1) * HSH]],
                outs=[tfull[half * HTOT:(half + 1) * HTOT]])
        for b in range(NBLK):
            dense_block(0, b)
            if b == NBLK // 2 - 1:
                ag(tab_sh[0], tab_full[0], 0)
        ag(tab_sh[0], tab_full[0], 1)

        # ---------------- layer 1 edge (+ layer 2 dense interleaved)
        def after1(b, hn16):
            tp = pst.tile([P, P], F16, tag="tr")
            nc.tensor.transpose(out=tp, in_=hn16[:, 0:P], identity=ident16)
            nc.scalar.activation(out=hT[:, 2 * b, :], in_=tp, func=COPYF)
            tp2 = pst.tile([P, P], F16, tag="tr")
            nc.tensor.transpose(out=tp2, in_=hn16[:, P:HC], identity=ident16)
            nc.scalar.activation(out=hT[:, 2 * b + 1, :], in_=tp2, func=COPYF)
            dense_block(1, b)
        def mid1():
            ag(tab_sh[1], tab_full[1], 0)
        edge12(0, b1r, after1, mid1)
        ag(tab_sh[1], tab_full[1], 1)

        # ---------------- layer 2 edge (+ layer 3 dense inline)
        def after2(b, hn16):
            t3 = small.tile([P, HC], F16, tag="t3")
            nc.vector.tensor_tensor(out=t3, in0=hn16, in1=w3r,
                                    op=mybir.AluOpType.mult)
            h3 = small.tile([P, 1], F32, tag="h3")
            nc.vector.tensor_reduce(out=h3, in_=t3, axis=mybir.AxisListType.X,
                                    op=mybir.AluOpType.add)
            row3 = small.tile([P, 2], F16, tag="row3")
            nc.scalar.activation(out=row3[:, 0:1], in_=h3, func=COPYF)
            nc.vector.tensor_tensor(out=row3[:, 1:2], in0=h3, in1=sc3[:, 0:1],
                                    op=mybir.AluOpType.mult)
            nc.sync.dma_start(
                out=tab3_sh.rearrange("(b p) a -> p b a", p=P)[:, b, 0:2],
                in_=row3)
            nc.vector.tensor_tensor(out=adl3[:, b, :], in0=h3, in1=sc3[:, 1:2],
                                    op=mybir.AluOpType.mult)
        def mid2():
            ag(tab3_sh, tab3_full, 0)
        edge12(1, b2r, after2, mid2)
        ag(tab3_sh, tab3_full, 1)

        # ---------------- layer 3 edge
        for b in range(NBLK):
                S, St = load_s(b)
                g3 = gpool.tile([P, T, 128], F16, tag="g3")
                iw = T * 8
                nc.gpsimd.dma_gather(g3, tab3_full[:],
                                     srcg[:, b * iw:(b + 1) * iw],
                                     T * P, nidx_reg, 128,
                                     single_packet=False)
                d3 = psa.tile([P, T, HEADS], F32, tag="adx")
                for t in range(T):
                    nc.tensor.matmul(d3[:, t, 0:1], St[:, t, :], adl3[:, b, :],
                                     start=True, stop=True)
                e3 = small.tile([P, T, 1], F32, tag="e3")
                nc.vector.tensor_tensor(out=e3, in0=g3[:, :, 1:2],
                                        in1=d3[:, :, 0:1],
                                        op=mybir.AluOpType.add)
                lk3 = small.tile([P, T, 1], F32, tag="lk3")
                nc.vector.tensor_scalar_mul(lk3, e3, NEG)
                nc.vector.tensor_tensor(out=lk3, in0=lk3, in1=e3,
                                        op=mybir.AluOpType.max)
                ex3 = small.tile([P, T, 1], F32, tag="ex3")
                nc.scalar.activation(out=ex3, in_=lk3, func=EXPF)
                m3 = small.tile([P, T, 2], F16, tag="m3")
                nc.vector.tensor_tensor(out=m3[:, :, 0:1], in0=ex3,
                                        in1=g3[:, :, 0:1],
                                        op=mybir.AluOpType.mult)
                nc.scalar.activation(out=m3[:, :, 1:2], in_=ex3, func=COPYF)
                ps3f = pse.tile([P, ROWW], F32, tag="agg")
                ps3 = ps3f[:, 0:2]
                for t in range(T):
                    nc.tensor.matmul(ps3, S[:, t, :], m3[:, t, :],
                                     start=(t == 0), stop=(t == T - 1))
                den3 = small.tile([P, 1], F32, tag="den3")
                nc.scalar.activation(out=den3, in_=ps3[:, 1:2], func=COPYF,
                                     bias=1e-30)
                rec3 = small.tile([P, 1], F32, tag="rec3")
                nc.vector.reciprocal(out=rec3, in_=den3)
                nc.vector.tensor_tensor(out=outsb[:, b:b + 1], in0=ps3[:, 0:1],
                                        in1=rec3, op=mybir.AluOpType.mult)
        nc.vector.tensor_tensor(out=outsb, in0=outsb,
                                in1=bass.AP(tensor=sc3.tensor,
                                            offset=sc3[:, 2:3].offset,
                                            ap=[list(sc3.ap[0]), [0, NBLK]]),
                                op=mybir.AluOpType.add)
        nc.sync.dma_start(out=out_p[:], in_=outsb)

    mybir.codegen_inst_isa_subclasses(nc)
    return nc


_CACHE = {}


def kernel(x, edge_index, W1, a_src1, a_dst1, b1, W2, a_src2, a_dst2, b2,
           W3, a_src3, a_dst3, b3):
    T, gslot, node_of_slot, srcg, s_mat, st_mat = _preprocess(np.asarray(edge_index))

    wa1 = _aug_weights(np.asarray(W1, np.float32), np.asarray(a_src1, np.float32),
                       np.asarray(a_dst1, np.float32), HEADS, HID)
    wa2 = _aug_weights(np.asarray(W2, np.float32), np.asarray(a_src2, np.float32),
                       np.asarray(a_dst2, np.float32), HEADS, HID)
    w3 = np.asarray(W3, np.float32).reshape(1, HC).astype(np.float16)
    sc3 = np.array([[float(np.asarray(a_src3).reshape(-1)[0]),
                     float(np.asarray(a_dst3).reshape(-1)[0]),
                     float(np.asarray(b3).reshape(-1)[0]), 0.0]], np.float32)
    b1r = np.asarray(b1, np.float32).reshape(1, HC)
    b2r = np.asarray(b2, np.float32).reshape(1, HC)

    x = np.asarray(x, np.float32)
    in_maps = []
    for c in range(NCORES):
        sl = slice(c * SLOTS, (c + 1) * SLOTS)
        nos = node_of_slot[sl]
        xs = np.zeros((SLOTS, IN_DIM), np.float32)
        valid = nos >= 0
        xs[valid] = x[nos[valid]]
        xT = xs.T.astype(np.float16).reshape(P, SLOTS)
        in_maps.append({
            "xT_in": xT,
            "srcg_in": srcg[c],
            "s_in": np.ascontiguousarray(s_mat[c].transpose(1, 0, 2)).reshape(P, -1),
            "st_in": np.ascontiguousarray(st_mat[c].transpose(1, 0, 2)).reshape(P, -1),
            "wa1_in": wa1, "wa2_in": wa2, "w3_in": w3, "sc3_in": sc3,
            "b1_in": b1r, "b2_in": b2r,
            "ident_in": np.eye(P, dtype=np.float16),
        })

    if T not in _CACHE:
        _CACHE[T] = _build(T)
    nc = _CACHE[T]
    res = run_bass_kernel_spmd(nc, in_maps, list(range(NCORES)))

    out = np.empty(N_NODES, np.float32)
    for c in range(NCORES):
        o = res.results[c]["out_p"]
        flat = o.T.reshape(-1)
        nos = node_of_slot[c * SLOTS:(c + 1) * SLOTS]
        valid = nos >= 0
        out[nos[valid]] = flat[valid]
    return out
